# revision 44
# baseline (speedup 1.0000x reference)
"""Multi-head attention forward kernel for Trainium2 (8 NeuronCores).

Problem: B=2, N=2048, C=1024, H=16 heads, head_dim=64.
    q = x @ Wq.T + bq  (same for k, v)
    out = softmax(q k^T / sqrt(C)) v       (per head), re-merged to [B, N, C]

Sharding: core = (batch b, head-group g): b = core // 4, g = core % 4.
Each core computes 4 heads of one batch element. No collectives needed --
outputs are disjoint; host gathers and finishes with a cheap epilogue
(normalize by the row-sums, add the V bias, transpose).

v2 design notes (measured atoms from microbench):
  - Any 512-col MM "slot" (single, row-packed pair, col-packed pair) paces
    at ~259 ns back-to-back; LDWEIGHTS hides completely. PE total ~125 us.
  - ACT exp from PSUM runs ~1.18 ns/elem regardless of op size ->
    ACT busy floor ~155 us/core. ACT is THE bottleneck; everything else
    is scheduled to keep the exp stream gapless.
  - st ring-3 (stp bufs=3, 6 PSUM banks) so QK can run 2 steps ahead of
    exp; o_ps 1 bank; proj+sums share 1 bank (ppsum bufs=1).
  - V bias is softmax-invariant additive on the output -> applied on host;
    V evacuates via plain tensor_copy (cheaper DVE).
  - out_o shipped as bf16 (halves out-DMA, 2x DVE copy mode).
  - Granular input DMA (w's first, then xt in nb-major 512-col slices) so
    the first exp fires ~8 us in; V blocks 0-2 emitted in the prologue,
    V[s+2] per qb0 step s, kt/qt projection blocks as PE filler inside the
    ACT-bound window (emission order = scheduler priority).
Outputs: out_o [2, 128, N] bf16 (pair, head-major O^T rows, queries),
         out_s [2, 2, N] f32   (pair, head, query sums).
"""

import os
import sys

import ml_dtypes
import numpy as np

for _p in ("/opt/trn_rl_repo",):
    if _p not in sys.path:
        sys.path.insert(0, _p)

import concourse.bass as bass  # noqa: E402
import concourse.tile as tile  # noqa: E402
from concourse import bacc, mybir  # noqa: E402
from concourse.bass_utils import run_bass_kernel_spmd  # noqa: E402

N = 2048  # sequence length
C = 1024  # model dim
D = 64  # head dim
NH = 4  # heads per core
HD = NH * D  # 256 output channels per core
NCORES = 8
KB = N // 128  # 16 key chunks of 128
QB = N // 512  # 4 query blocks of 512
KC = C // 128  # 8 contraction chunks for projections
SCALE = 1.0 / 32.0  # 1 / sqrt(C)

F32 = mybir.dt.float32
BF16 = mybir.dt.bfloat16
FP16 = mybir.dt.float16


def build_kernel(tc, xt, wqt, wkt, wvt, out_o, out_s):
    nc = tc.nc
    Exp = mybir.ActivationFunctionType.Exp

    with (
        tc.tile_pool(name="res", bufs=1) as res,
        tc.tile_pool(name="ppsum", bufs=1, space="PSUM") as ppsum,
        tc.tile_pool(name="stp", bufs=3, space="PSUM") as stp,
        tc.tile_pool(name="opp", bufs=1, space="PSUM") as opp,
        tc.tile_pool(name="ptp", bufs=10) as ptp,
        tc.tile_pool(name="otp", bufs=2) as otp,
        tc.tile_pool(name="ssp", bufs=2) as ssp,
    ):
        # ---- resident SBUF tensors ----
        # W layout [128, 2 bias cols + (m, k, d) m-major weights]: the two
        # bias columns ride inside the same contiguous DMA (a standalone
        # [128,1] bias DMA is a 4-byte-packet storm that stalls the queue),
        # and the m-major order lets each head-pair half load separately.
        wq_flat = res.tile([128, 2 + 2048], BF16, tag="wq", name="wq")
        wk_flat = res.tile([128, 2 + 2048], BF16, tag="wk", name="wk")
        wv_flat = res.tile([128, 2048], BF16, tag="wv", name="wv")
        xt_all = res.tile([128, KC, N], BF16, tag="xt", name="xt")
        xt_sb = [xt_all[:, k, :] for k in range(KC)]
        wq_m = wq_flat[:, 2:].rearrange("p (m k d) -> p m k d", m=2, k=KC)
        wk_m = wk_flat[:, 2:].rearrange("p (m k d) -> p m k d", m=2, k=KC)
        wv_m = wv_flat.rearrange("p (m k d) -> p m k d", m=2, k=KC)
        bqf = res.tile([128, 2], F32, tag="bqf", name="bqf")
        bkf = res.tile([128, 2], F32, tag="bkf", name="bkf")
        bq_sb = [bqf[:, m : m + 1] for m in range(2)]
        bk_sb = [bkf[:, m : m + 1] for m in range(2)]
        qt_sb = [res.tile([128, N], BF16, tag=f"qt{m}", name=f"qt{m}") for m in range(2)]
        kt_sb = [res.tile([128, N], BF16, tag=f"kt{m}", name=f"kt{m}") for m in range(2)]
        v_sb = [res.tile([128, NH, D], FP16, tag=f"v{kb}", name=f"v{kb}") for kb in range(KB)]
        ones_sb = res.tile([128, 1], FP16, tag="ones", name="ones")
        warm_sb = res.tile([1, 2], F32, tag="warm", name="warm")
        # pair-0 strip-mode state: resident softmax-sum parity accumulators
        # (all 4 qbs live at once) and per-qb SBUF O accumulators that
        # collect 4-kb strip partials from the single o PSUM bank.
        ssum_sb = [
            [res.tile([128, 2, 512], FP16, tag=f"ss{q}{j}", name=f"ss{q}{j}")
             for j in range(2)]
            for q in range(QB)
        ]
        o_acc = [res.tile([128, 512], F32, tag=f"oa{q}", name=f"oa{q}")
                 for q in range(QB)]

        # ---- input DMAs, ordered by consumer deadline (HBM bandwidth is
        # shared by all 8 cores; the whole input set takes tens of us).
        # FEW, BIG descriptors: each DMA_DIRECT2D trigger costs ~600 ns on
        # the sync queue and >.30 queued descriptors stall on ring space,
        # delaying later transfers by ~10 us (and the resulting PE idle
        # re-throttles HAM to K=4/8).  Partition lines stay >=1KB. ----
        # Two hardware DGE queues (SP + Activation) run in parallel: the
        # critical xt nb0/nb1 stream goes on the scalar queue (idle until
        # the first exp anyway) while the W stream runs on sync, halving
        # the serial prefix before the first exp.
        xtr = xt.rearrange("(k p) n -> p k n", p=128)
        nc.scalar.dma_start(out=xt_all[:, 0:4, 0:512], in_=xtr[:, 0:4, 0:512])
        nc.scalar.dma_start(out=xt_all[:, 4:8, 0:512], in_=xtr[:, 4:8, 0:512])
        nc.scalar.dma_start(out=xt_all[:, :, 512:1024], in_=xtr[:, :, 512:1024])
        for half in range(2):
            lo, hi = 2 + half * 512, 2 + (half + 1) * 512
            nc.sync.dma_start(out=wq_flat[:, (0 if half == 0 else lo) : hi],
                              in_=wqt[:, (0 if half == 0 else lo) : hi])
            nc.sync.dma_start(out=wk_flat[:, (0 if half == 0 else lo) : hi],
                              in_=wkt[:, (0 if half == 0 else lo) : hi])
        nc.sync.dma_start(out=wv_flat[:, 0:1024], in_=wvt[:, 0:1024])
        for nb in range(2, QB):
            nsl = slice(nb * 512, (nb + 1) * 512)
            nc.sync.dma_start(out=xt_all[:, :, nsl], in_=xtr[:, :, nsl])
        # pair-1 halves: needed only from pair0-qb2 onwards
        nc.sync.dma_start(out=wq_flat[:, 2 + 1024 :], in_=wqt[:, 2 + 1024 :])
        nc.sync.dma_start(out=wk_flat[:, 2 + 1024 :], in_=wkt[:, 2 + 1024 :])
        nc.sync.dma_start(out=wv_flat[:, 1024:], in_=wvt[:, 1024:])
        nc.vector.memset(ones_sb[:], 1.0)
        # widen the in-DMA bf16 bias columns to f32 for tensor_scalar
        nc.vector.tensor_copy(out=bqf[:], in_=wq_flat[:, 0:2])
        nc.vector.tensor_copy(out=bkf[:], in_=wk_flat[:, 0:2])
        # warm up the ACT exp table while DMAs land
        nc.vector.memset(warm_sb[:], 0.0)
        nc.scalar.activation(out=warm_sb[:, 0:1], in_=warm_sb[:, 1:2], func=Exp)

        # Projection blocks are emitted in HALVES (4 contraction chunks
        # each) on consecutive steps so a block never overflows a single
        # ACT-bound step window and stalls the exp chain through the PE
        # FIFO.  The live PSUM tile is kept in `pending` between halves.
        pending = {}

        def proj_qk_half(which, m, nb, part):
            key = (which, m, nb)
            nsl = slice(nb * 512, (nb + 1) * 512)
            if part == 0:
                pending[key] = ppsum.tile([128, 512], F32, tag="qkps", name="qkps")
            ps = pending[key]
            w_m = wq_m if which == "q" else wk_m
            for k in range(part * 4, part * 4 + 4):
                nc.tensor.matmul(
                    out=ps[:],
                    lhsT=w_m[:, m, k, :],
                    rhs=xt_sb[k][:, nsl],
                    start=(k == 0),
                    stop=(k == KC - 1),
                )
            if part == 1:
                b_sb = (bq_sb if which == "q" else bk_sb)[m]
                t_sb = (qt_sb if which == "q" else kt_sb)[m]
                nc.vector.tensor_scalar_add(out=t_sb[:, nsl], in0=ps[:], scalar1=b_sb[:])
                del pending[key]

        def proj_qk_block(which, m, nb):
            proj_qk_half(which, m, nb, 0)
            proj_qk_half(which, m, nb, 1)

        def proj_v_block(kb, half):
            # one head-pair's V columns: pair-0's V is needed from the very
            # first PV steps, pair-1's only once pair 1 starts -> split so
            # qb0 carries half the V-projection load.
            vps = ppsum.tile([128, 128], F32, tag="qkps", name="vps")
            for k in range(KC):
                nc.tensor.matmul(
                    out=vps[:],
                    lhsT=xt_sb[k][:, kb * 128 : (kb + 1) * 128],
                    rhs=wv_m[:, half, k, :],
                    start=(k == 0),
                    stop=(k == KC - 1),
                )
            # V bias is applied on the host (softmax-invariant): plain copy.
            nc.vector.tensor_copy(
                out=v_sb[kb][:, 2 * half : 2 * half + 2, :],
                in_=vps[:].rearrange("p (h d) -> p h d", h=2),
            )

        def attn_strip(p, filler_map):
            """Pair-0 attention in 4-kb-strip x qb cells, ordered so early
            cells only consume xt nb0/nb1 -- the later x slices stream in
            behind the compute instead of stalling the exp chain."""
            cells = [(0, 0), (1, 0), (0, 1), (1, 1), (2, 0), (2, 1), (0, 2),
                     (1, 2), (2, 2), (3, 0), (0, 3), (3, 1), (1, 3), (3, 2),
                     (2, 3), (3, 3)]
            prev = None       # pending PV(+evac) closure from the last step
            fin_pending = None
            for (j, i) in cells:
                qb = i
                qsl = slice(qb * 512, (qb + 1) * 512)
                o_ps = opp.tile([128, 512], F32, tag="o", name="o")
                for t in range(4):
                    kb = 4 * j + t
                    ksl = slice(kb * 128, (kb + 1) * 128)
                    st = stp.tile([128, 2, 512], F32, tag="st", name="st")
                    for h in range(2):
                        hsl = slice(h * D, (h + 1) * D)
                        nc.tensor.matmul(
                            out=st[:, h, :],
                            lhsT=kt_sb[p][hsl, ksl],
                            rhs=qt_sb[p][hsl, qsl],
                            start=True,
                            stop=True,
                        )
                    pt = ptp.tile([128, 2, 512], FP16, tag="pt", name="pt")
                    nc.scalar.activation(out=pt[:], in_=st[:], func=Exp, scale=SCALE)
                    if t == 1 and fin_pending is not None:
                        fin_pending()
                        fin_pending = None
                    if prev is not None:
                        prev()
                        prev = None

                    def step_pv(kb=kb, qb=qb, pt=pt, o_ps=o_ps, t=t, j=j):
                        for h in range(2):
                            nc.tensor.matmul(
                                out=o_ps[h * D : (h + 1) * D, :],
                                lhsT=v_sb[kb][:, 2 * p + h, :],
                                rhs=pt[:, h, :],
                                start=(t == 0),
                                stop=(t == 3),
                                tile_position=(0, h * D),
                                skip_group_check=True,
                            )
                        # evac first: the next cell's o alloc waits on it,
                        # so it must not queue behind the ssum add on DVE
                        if t == 3:
                            if j == 0:
                                nc.vector.tensor_copy(out=o_acc[qb][:], in_=o_ps[:])
                            else:
                                nc.vector.tensor_add(
                                    out=o_acc[qb][:], in0=o_acc[qb][:], in1=o_ps[:]
                                )
                        sj = ssum_sb[qb][kb % 2]
                        if kb < 2:
                            nc.vector.tensor_copy(out=sj[:], in_=pt[:])
                        else:
                            nc.vector.tensor_add(out=sj[:], in0=sj[:], in1=pt[:])

                    prev = step_pv
                    for fn in filler_map.get((qb, kb), ()):
                        fn()

                if j == 3:
                    def strip_fin(qb=qb, qsl=qsl):
                        s_ps = ppsum.tile([33, 512], F32, tag="qkps", name="sps")
                        for h in range(2):
                            for j2 in range(2):
                                nc.tensor.matmul(
                                    out=s_ps[32 * h : 32 * h + 1, :],
                                    lhsT=ones_sb[:],
                                    rhs=ssum_sb[qb][j2][:, h, :],
                                    start=(j2 == 0),
                                    stop=(j2 == 1),
                                    tile_position=(0, 32 * h),
                                    skip_group_check=True,
                                )
                        ss = otp.tile([33, 512], F32, tag="ss", name="ss")
                        for h in range(2):
                            nc.vector.tensor_copy(
                                out=ss[32 * h : 32 * h + 1, :],
                                in_=s_ps[32 * h : 32 * h + 1, :],
                            )
                        ss_view = bass.AP(
                            tensor=ss.tensor, offset=ss.offset,
                            ap=[[32 * ss.ap[0][0], 2]] + list(ss.ap[1:]),
                        )
                        nc.sync.dma_start(out=out_s[p, :, qsl], in_=ss_view)
                        ot = otp.tile([128, 512], BF16, tag="ot", name="ot")
                        nc.vector.tensor_copy(out=ot[:], in_=o_acc[qb][:])
                        nc.sync.dma_start(out=out_o[p, :, qsl], in_=ot[:])

                    fin_pending = strip_fin
            prev()
            return fin_pending

        def attn(p, filler_hook=None, carry_fin=None):
            fin_pending = carry_fin
            for qb in range(QB):
                qsl = slice(qb * 512, (qb + 1) * 512)
                # both heads' O^T col-packed: head h at partitions h*64..
                o_ps = opp.tile([128, 512], F32, tag="o", name="o")
                # running sums of P^T chunks (softmax denominators): two
                # fp16 parity accumulators keep the DVE in its fast 2-byte
                # mode and halve the accumulation depth.
                ssum = [
                    ssp.tile([128, 2, 512], FP16, tag=f"ssum{j}", name=f"ssum{j}")
                    for j in range(2)
                ]

                def emit_pv(args):
                    kb, pt = args
                    for h in range(2):
                        nc.tensor.matmul(
                            out=o_ps[h * D : (h + 1) * D, :],
                            lhsT=v_sb[kb][:, 2 * p + h, :],
                            rhs=pt[:, h, :],
                            start=(kb == 0),
                            stop=(kb == KB - 1),
                            tile_position=(0, h * D),
                            skip_group_check=True,
                        )
                    sj = ssum[kb % 2]
                    if kb < 2:
                        nc.vector.tensor_copy(out=sj[:], in_=pt[:])
                    else:
                        nc.vector.tensor_add(out=sj[:], in0=sj[:], in1=pt[:])

                # Per step: QK -> exp -> PV(prev) -> fillers.  The QK/exp
                # chain leads; PV lags one step (pt pool decouples); filler
                # projection blocks absorb the PE slack under the ACT-bound
                # exp stream.
                prev = None
                for kb in range(KB):
                    ksl = slice(kb * 128, (kb + 1) * 128)
                    # st layout [128 keys, head, 512 q] fp32: head h
                    # occupies its own PSUM bank; ring-3 lets QK run ~2
                    # steps ahead of the exp stream.
                    st = stp.tile([128, 2, 512], F32, tag="st", name="st")
                    for h in range(2):
                        hsl = slice(h * D, (h + 1) * D)
                        nc.tensor.matmul(
                            out=st[:, h, :],
                            lhsT=kt_sb[p][hsl, ksl],
                            rhs=qt_sb[p][hsl, qsl],
                            start=True,
                            stop=True,
                        )
                    pt = ptp.tile([128, 2, 512], FP16, tag="pt", name="pt")
                    nc.scalar.activation(out=pt[:], in_=st[:], func=Exp, scale=SCALE)
                    if kb == 1 and fin_pending is not None:
                        fin_pending()
                        fin_pending = None
                    if prev is not None:
                        emit_pv(prev)
                    prev = (kb, pt)
                    if filler_hook is not None:
                        filler_hook(qb, kb)
                emit_pv(prev)

                # Finalize (partition-reduce the running sums with
                # ones-vector matmuls -- both parity accumulators accumulate
                # into the same PSUM row, head h at PSUM partition 32*h --
                # then evacuate sums + O and DMA out).  Deferred into the
                # next qb's step 1 so it never sits ahead of the next qb's
                # QK chain in the engine FIFOs.
                def finalize(qb=qb, qsl=qsl, o_ps=o_ps, ssum=ssum):
                    s_ps = ppsum.tile([33, 512], F32, tag="qkps", name="sps")
                    for h in range(2):
                        for j in range(2):
                            nc.tensor.matmul(
                                out=s_ps[32 * h : 32 * h + 1, :],
                                lhsT=ones_sb[:],
                                rhs=ssum[j][:, h, :],
                                start=(j == 0),
                                stop=(j == 1),
                                tile_position=(0, 32 * h),
                                skip_group_check=True,
                            )
                    ss = otp.tile([33, 512], F32, tag="ss", name="ss")
                    for h in range(2):
                        nc.vector.tensor_copy(
                            out=ss[32 * h : 32 * h + 1, :],
                            in_=s_ps[32 * h : 32 * h + 1, :],
                        )
                    ss_view = bass.AP(
                        tensor=ss.tensor, offset=ss.offset,
                        ap=[[32 * ss.ap[0][0], 2]] + list(ss.ap[1:]),
                    )
                    nc.sync.dma_start(out=out_s[p, :, qsl], in_=ss_view)
                    ot = otp.tile([128, 512], BF16, tag="ot", name="ot")
                    nc.vector.tensor_copy(out=ot[:], in_=o_ps[:])
                    nc.sync.dma_start(out=out_o[p, :, qsl], in_=ot[:])

                fin_pending = finalize
            return fin_pending

        def proj_qk_first():
            qps = ppsum.tile([128, 512], F32, tag="qkps", name="qkps")
            kps = ppsum.tile([128, 512], F32, tag="qkps", name="qkps")
            for k in range(KC):
                for w_m, ps in ((wq_m, qps), (wk_m, kps)):
                    nc.tensor.matmul(
                        out=ps[:],
                        lhsT=w_m[:, 0, k, :],
                        rhs=xt_sb[k][:, 0:512],
                        start=(k == 0),
                        stop=(k == KC - 1),
                    )
            nc.vector.tensor_scalar_add(out=qt_sb[0][:, 0:512], in0=qps[:], scalar1=bq_sb[0][:])
            nc.vector.tensor_scalar_add(out=kt_sb[0][:, 0:512], in0=kps[:], scalar1=bk_sb[0][:])

        # Filler schedule.  Pair-0 (strip mode) keys fillers by the actual
        # (qb, kb) of each step; blocks are placed after their input DMA
        # lands (nb1 ~30us, nb2 ~38, nb3 ~46, wv-m1 ~52) and at least one
        # cell before their consumer.
        def sched(table, qb, kb):
            for (q, s), fn in table:
                if q == qb and s == kb:
                    fn()

        def V0(kb):
            return lambda: proj_v_block(kb, 0)

        def V1(kb):
            return lambda: proj_v_block(kb, 1)

        def PQ(which, m, nb, part):
            return lambda: proj_qk_half(which, m, nb, part)

        p0_map = {
            (0, 0): [V0(1)], (0, 1): [V0(2)],
            (0, 2): [PQ("k", 0, 1, 0)], (0, 3): [PQ("k", 0, 1, 1), V0(3)],
            (0, 4): [PQ("q", 0, 1, 0), V0(4)], (0, 5): [PQ("q", 0, 1, 1), V0(5)],
            (0, 6): [V0(6)], (0, 7): [V0(7)],
            (1, 4): [PQ("k", 0, 2, 0)], (1, 5): [PQ("k", 0, 2, 1)],
            (0, 8): [V0(8)], (0, 9): [V0(9)], (0, 10): [V0(10)], (0, 11): [V0(11)],
            (1, 8): [PQ("q", 0, 2, 0)], (1, 9): [PQ("q", 0, 2, 1)],
            (2, 0): [PQ("q", 0, 3, 0)], (2, 1): [PQ("q", 0, 3, 1)],
            (2, 2): [PQ("k", 0, 3, 0)], (2, 3): [PQ("k", 0, 3, 1)],
            (2, 4): [V1(0)], (2, 6): [V1(1)],
            (2, 8): [V1(2)], (2, 10): [V1(3)],
            (0, 12): [V0(12)], (0, 13): [V0(13)], (0, 14): [V0(14)], (0, 15): [V0(15)],
            (3, 0): [V1(4)], (3, 1): [V1(5)],
            (3, 2): [PQ("k", 1, 0, 0)], (3, 3): [PQ("k", 1, 0, 1)],
            (1, 12): [V1(6)], (1, 13): [V1(7)], (1, 14): [V1(8)], (1, 15): [V1(9)],
            (3, 4): [V1(10)], (3, 7): [V1(11)],
            (3, 5): [PQ("q", 1, 0, 0)], (3, 6): [PQ("q", 1, 0, 1)],
            (2, 12): [PQ("k", 1, 1, 0)], (2, 13): [PQ("k", 1, 1, 1)],
            (2, 14): [V1(12)], (2, 15): [V1(13)],
            (3, 8): [V1(14)], (3, 9): [V1(15)],
        }

        p1_table = [
            ((0, 2), lambda: proj_qk_half("k", 1, 2, 0)),
            ((0, 3), lambda: proj_qk_half("k", 1, 2, 1)),
            ((0, 6), lambda: proj_qk_half("k", 1, 3, 0)),
            ((0, 7), lambda: proj_qk_half("k", 1, 3, 1)),
            ((0, 10), lambda: proj_qk_half("q", 1, 1, 0)),
            ((0, 11), lambda: proj_qk_half("q", 1, 1, 1)),
            ((1, 1), lambda: proj_qk_half("q", 1, 2, 0)),
            ((1, 2), lambda: proj_qk_half("q", 1, 2, 1)),
            ((1, 7), lambda: proj_qk_half("q", 1, 3, 0)),
            ((1, 8), lambda: proj_qk_half("q", 1, 3, 1)),
        ]

        proj_qk_first()
        proj_v_block(0, 0)
        fin = attn_strip(0, p0_map)
        fin = attn(1, filler_hook=lambda qb, kb: sched(p1_table, qb, kb),
                   carry_fin=fin)
        fin()


def build_nc():
    nc = bacc.Bacc(
        "TRN2",
        target_bir_lowering=False,
        debug=False,
        num_devices=NCORES,
        enable_partition_id=False,
    )
    xt = nc.dram_tensor("xt", [C, N], BF16, kind="ExternalInput").ap()
    wqt = nc.dram_tensor("wqt", [128, 2 + 2048], BF16, kind="ExternalInput").ap()
    wkt = nc.dram_tensor("wkt", [128, 2 + 2048], BF16, kind="ExternalInput").ap()
    wvt = nc.dram_tensor("wvt", [128, 2048], BF16, kind="ExternalInput").ap()
    out_o = nc.dram_tensor("out_o", [2, 128, N], BF16, kind="ExternalOutput").ap()
    out_s = nc.dram_tensor("out_s", [2, 2, N], F32, kind="ExternalOutput").ap()

    with tile.TileContext(nc) as tc:
        build_kernel(tc, xt, wqt, wkt, wvt, out_o, out_s)
    nc.compile()
    return nc


def _w_prep(w, sl, bias=None):
    # [HD-slice, C] weight -> SBUF-ready m-major [128, (2 kd)]: element
    # (c=k*128+p, h=m*128+j) -> [p, m, k, j], flattened; with the two bias
    # columns (bias[m*128+p] on partition p) prepended when given.
    wt = np.asarray(w, np.float32)[sl, :].T  # [C, HD]
    wt = wt.reshape(KC, 128, 2, 128).transpose(1, 2, 0, 3).reshape(128, 2048)
    if bias is None:
        return np.ascontiguousarray(wt).astype(ml_dtypes.bfloat16)
    b = np.asarray(bias, np.float32)[sl].reshape(2, 128).T  # [128, 2]
    return np.ascontiguousarray(np.concatenate([b, wt], axis=1)).astype(
        ml_dtypes.bfloat16
    )


def shard_inputs(inputs):
    x = np.asarray(inputs["x"], np.float32)
    in_maps = []
    for core in range(NCORES):
        b, g = core // 4, core % 4
        sl = slice(g * HD, (g + 1) * HD)
        in_maps.append(
            {
                "xt": np.ascontiguousarray(x[b].T).astype(ml_dtypes.bfloat16),
                "wqt": _w_prep(inputs["Wq"], sl, inputs["bq"]),
                "wkt": _w_prep(inputs["Wk"], sl, inputs["bk"]),
                "wvt": _w_prep(inputs["Wv"], sl),
            }
        )
    return in_maps


def assemble(results, inputs, B=2):
    bv = np.asarray(inputs["bv"], np.float32)
    out = np.zeros((B, N, C), np.float32)
    for core in range(NCORES):
        b, g = core // 4, core % 4
        oo = np.asarray(results[core]["out_o"], np.float32)  # [2, 128, N]
        os_ = np.asarray(results[core]["out_s"], np.float32)  # [2, 2, N]
        o = oo.reshape(2, 2, D, N)  # [pair, head, d, n]
        on = o / os_[:, :, None, :]
        # [pair, head, d, n] -> [n, pair*2*D + head*D + d], + host-side bv
        out[b, :, g * HD : (g + 1) * HD] = (
            on.transpose(3, 0, 1, 2).reshape(N, HD) + bv[g * HD : (g + 1) * HD]
        )
    return out


_NC_CACHE = None


def _get_nc():
    global _NC_CACHE
    if _NC_CACHE is None:
        _NC_CACHE = build_nc()
    return _NC_CACHE


def kernel(**inputs):
    nc = _get_nc()
    in_maps = shard_inputs(inputs)
    res = run_bass_kernel_spmd(
        nc,
        in_maps,
        core_ids=list(range(NCORES)),
        trace=bool(int(os.environ.get("KERNEL_TRACE", "0"))),
    )
    return assemble(res.results, inputs, B=int(np.asarray(inputs["x"]).shape[0]))


# revision 45
# speedup vs baseline: 1.1696x; 1.1696x over previous
"""Multi-head attention forward kernel for Trainium2 (8 NeuronCores).

Problem: B=2, N=2048, C=1024, H=16 heads, head_dim=64.
    q = x @ Wq.T + bq  (same for k, v)
    out = softmax(q k^T / sqrt(C)) v       (per head), re-merged to [B, N, C]

Sharding: core = (batch b, head-group g): b = core // 4, g = core % 4.
Each core computes 4 heads of one batch element. No collectives needed --
outputs are disjoint; host gathers and finishes with a cheap epilogue
(normalize by the row-sums, add the V bias, transpose).

v2 design notes (measured atoms from microbench):
  - Any 512-col MM "slot" (single, row-packed pair, col-packed pair) paces
    at ~259 ns back-to-back; LDWEIGHTS hides completely. PE total ~125 us.
  - ACT exp from PSUM runs ~1.18 ns/elem regardless of op size ->
    ACT busy floor ~155 us/core. ACT is THE bottleneck; everything else
    is scheduled to keep the exp stream gapless.
  - st ring-3 (stp bufs=3, 6 PSUM banks) so QK can run 2 steps ahead of
    exp; o_ps 1 bank; proj+sums share 1 bank (ppsum bufs=1).
  - V bias is softmax-invariant additive on the output -> applied on host;
    V evacuates via plain tensor_copy (cheaper DVE).
  - out_o shipped as bf16 (halves out-DMA, 2x DVE copy mode).
  - Granular input DMA (w's first, then xt in nb-major 512-col slices) so
    the first exp fires ~8 us in; V blocks 0-2 emitted in the prologue,
    V[s+2] per qb0 step s, kt/qt projection blocks as PE filler inside the
    ACT-bound window (emission order = scheduler priority).
Outputs: out_o [2, 128, N] bf16 (pair, head-major O^T rows, queries),
         out_s [2, 2, N] f32   (pair, head, query sums).
"""

import os
import sys

import ml_dtypes
import numpy as np

for _p in ("/opt/trn_rl_repo",):
    if _p not in sys.path:
        sys.path.insert(0, _p)

import concourse.bass as bass  # noqa: E402
import concourse.tile as tile  # noqa: E402
from concourse import bacc, mybir  # noqa: E402
from concourse.bass_utils import run_bass_kernel_spmd  # noqa: E402

N = 2048  # sequence length
C = 1024  # model dim
D = 64  # head dim
NH = 4  # heads per core
HD = NH * D  # 256 output channels per core
NCORES = 8
KB = N // 128  # 16 key chunks of 128
QB = N // 512  # 4 query blocks of 512
KC = C // 128  # 8 contraction chunks for projections
SCALE = 1.0 / 32.0  # 1 / sqrt(C)

F32 = mybir.dt.float32
BF16 = mybir.dt.bfloat16
FP16 = mybir.dt.float16


def build_kernel(tc, xt, wqt, wkt, wvt, out_o, out_s):
    nc = tc.nc
    Exp = mybir.ActivationFunctionType.Exp

    with (
        tc.tile_pool(name="res", bufs=1) as res,
        tc.tile_pool(name="ppsum", bufs=1, space="PSUM") as ppsum,
        tc.tile_pool(name="stp", bufs=3, space="PSUM") as stp,
        tc.tile_pool(name="opp", bufs=1, space="PSUM") as opp,
        tc.tile_pool(name="ptp", bufs=10) as ptp,
        tc.tile_pool(name="otp", bufs=2) as otp,
        tc.tile_pool(name="ssp", bufs=2) as ssp,
    ):
        # ---- resident SBUF tensors ----
        # W layout [128, 2 bias cols + (m, k, d) m-major weights]: the two
        # bias columns ride inside the same contiguous DMA (a standalone
        # [128,1] bias DMA is a 4-byte-packet storm that stalls the queue),
        # and the m-major order lets each head-pair half load separately.
        wq_flat = res.tile([128, 2 + 2048], BF16, tag="wq", name="wq")
        wk_flat = res.tile([128, 2 + 2048], BF16, tag="wk", name="wk")
        wv_flat = res.tile([128, 2048], BF16, tag="wv", name="wv")
        xt_all = res.tile([128, KC, N], BF16, tag="xt", name="xt")
        xt_sb = [xt_all[:, k, :] for k in range(KC)]
        wq_m = wq_flat[:, 2:].rearrange("p (m k d) -> p m k d", m=2, k=KC)
        wk_m = wk_flat[:, 2:].rearrange("p (m k d) -> p m k d", m=2, k=KC)
        wv_m = wv_flat.rearrange("p (m k d) -> p m k d", m=2, k=KC)
        bqf = res.tile([128, 2], F32, tag="bqf", name="bqf")
        bkf = res.tile([128, 2], F32, tag="bkf", name="bkf")
        bq_sb = [bqf[:, m : m + 1] for m in range(2)]
        bk_sb = [bkf[:, m : m + 1] for m in range(2)]
        qt_sb = [res.tile([128, N], BF16, tag=f"qt{m}", name=f"qt{m}") for m in range(2)]
        kt_sb = [res.tile([128, N], BF16, tag=f"kt{m}", name=f"kt{m}") for m in range(2)]
        v_sb = [res.tile([128, NH, D], FP16, tag=f"v{kb}", name=f"v{kb}") for kb in range(KB)]
        ones_sb = res.tile([128, 1], FP16, tag="ones", name="ones")
        warm_sb = res.tile([1, 2], F32, tag="warm", name="warm")
        # pair-0 strip-mode state: resident softmax-sum parity accumulators
        # (all 4 qbs live at once) and per-qb SBUF O accumulators that
        # collect 4-kb strip partials from the single o PSUM bank.
        ssum_sb = [
            [res.tile([128, 2, 512], FP16, tag=f"ss{q}{j}", name=f"ss{q}{j}")
             for j in range(2)]
            for q in range(QB)
        ]
        o_acc = [res.tile([128, 512], F32, tag=f"oa{q}", name=f"oa{q}")
                 for q in range(QB)]

        # ---- input DMAs, ordered by consumer deadline (HBM bandwidth is
        # shared by all 8 cores; the whole input set takes tens of us).
        # FEW, BIG descriptors: each DMA_DIRECT2D trigger costs ~600 ns on
        # the sync queue and >.30 queued descriptors stall on ring space,
        # delaying later transfers by ~10 us (and the resulting PE idle
        # re-throttles HAM to K=4/8).  Partition lines stay >=1KB. ----
        # Two hardware DGE queues (SP + Activation) run in parallel: the
        # critical xt nb0/nb1 stream goes on the scalar queue (idle until
        # the first exp anyway) while the W stream runs on sync, halving
        # the serial prefix before the first exp.
        xtr = xt.rearrange("(k p) n -> p k n", p=128)
        nc.scalar.dma_start(out=xt_all[:, 0:4, 0:512], in_=xtr[:, 0:4, 0:512])
        nc.scalar.dma_start(out=xt_all[:, 4:8, 0:512], in_=xtr[:, 4:8, 0:512])
        nc.scalar.dma_start(out=xt_all[:, :, 512:1024], in_=xtr[:, :, 512:1024])
        for half in range(2):
            lo, hi = 2 + half * 512, 2 + (half + 1) * 512
            nc.sync.dma_start(out=wq_flat[:, (0 if half == 0 else lo) : hi],
                              in_=wqt[:, (0 if half == 0 else lo) : hi])
            nc.sync.dma_start(out=wk_flat[:, (0 if half == 0 else lo) : hi],
                              in_=wkt[:, (0 if half == 0 else lo) : hi])
        nc.sync.dma_start(out=wv_flat[:, 0:1024], in_=wvt[:, 0:1024])
        for nb in range(2, QB):
            nsl = slice(nb * 512, (nb + 1) * 512)
            nc.sync.dma_start(out=xt_all[:, :, nsl], in_=xtr[:, :, nsl])
        # pair-1 halves: needed only from pair0-qb2 onwards
        nc.sync.dma_start(out=wq_flat[:, 2 + 1024 :], in_=wqt[:, 2 + 1024 :])
        nc.sync.dma_start(out=wk_flat[:, 2 + 1024 :], in_=wkt[:, 2 + 1024 :])
        nc.sync.dma_start(out=wv_flat[:, 1024:], in_=wvt[:, 1024:])
        nc.vector.memset(ones_sb[:], 1.0)
        # widen the in-DMA bf16 bias columns to f32 for tensor_scalar
        nc.vector.tensor_copy(out=bqf[:], in_=wq_flat[:, 0:2])
        nc.vector.tensor_copy(out=bkf[:], in_=wk_flat[:, 0:2])
        # warm up the ACT exp table while DMAs land
        nc.vector.memset(warm_sb[:], 0.0)
        nc.scalar.activation(out=warm_sb[:, 0:1], in_=warm_sb[:, 1:2], func=Exp)

        # Projection blocks are emitted in HALVES (4 contraction chunks
        # each) on consecutive steps so a block never overflows a single
        # ACT-bound step window and stalls the exp chain through the PE
        # FIFO.  The live PSUM tile is kept in `pending` between halves.
        pending = {}

        def proj_qk_half(which, m, nb, part):
            key = (which, m, nb)
            nsl = slice(nb * 512, (nb + 1) * 512)
            if part == 0:
                pending[key] = ppsum.tile([128, 512], F32, tag="qkps", name="qkps")
            ps = pending[key]
            w_m = wq_m if which == "q" else wk_m
            for k in range(part * 4, part * 4 + 4):
                nc.tensor.matmul(
                    out=ps[:],
                    lhsT=w_m[:, m, k, :],
                    rhs=xt_sb[k][:, nsl],
                    start=(k == 0),
                    stop=(k == KC - 1),
                )
            if part == 1:
                b_sb = (bq_sb if which == "q" else bk_sb)[m]
                t_sb = (qt_sb if which == "q" else kt_sb)[m]
                nc.vector.tensor_scalar_add(out=t_sb[:, nsl], in0=ps[:], scalar1=b_sb[:])
                del pending[key]

        def proj_qk_block(which, m, nb):
            proj_qk_half(which, m, nb, 0)
            proj_qk_half(which, m, nb, 1)

        def proj_v_block(kb, half):
            # one head-pair's V columns: pair-0's V is needed from the very
            # first PV steps, pair-1's only once pair 1 starts -> split so
            # qb0 carries half the V-projection load.
            vps = ppsum.tile([128, 128], F32, tag="qkps", name="vps")
            for k in range(KC):
                nc.tensor.matmul(
                    out=vps[:],
                    lhsT=xt_sb[k][:, kb * 128 : (kb + 1) * 128],
                    rhs=wv_m[:, half, k, :],
                    start=(k == 0),
                    stop=(k == KC - 1),
                )
            # V bias is applied on the host (softmax-invariant): plain copy.
            nc.vector.tensor_copy(
                out=v_sb[kb][:, 2 * half : 2 * half + 2, :],
                in_=vps[:].rearrange("p (h d) -> p h d", h=2),
            )

        def attn_strip(p, filler_map):
            """Pair-0 attention in 4-kb-strip x qb cells, ordered so early
            cells only consume xt nb0/nb1 -- the later x slices stream in
            behind the compute instead of stalling the exp chain."""
            cells = [(0, 0), (1, 0), (0, 1), (1, 1), (2, 0), (2, 1), (0, 2),
                     (1, 2), (2, 2), (3, 0), (0, 3), (3, 1), (1, 3), (3, 2),
                     (2, 3), (3, 3)]
            prev = None       # pending PV(+evac) closure from the last step
            fin_pending = None
            for (j, i) in cells:
                qb = i
                qsl = slice(qb * 512, (qb + 1) * 512)
                o_ps = opp.tile([128, 512], F32, tag="o", name="o")
                for t in range(4):
                    kb = 4 * j + t
                    ksl = slice(kb * 128, (kb + 1) * 128)
                    st = stp.tile([128, 2, 512], F32, tag="st", name="st")
                    for h in range(2):
                        hsl = slice(h * D, (h + 1) * D)
                        nc.tensor.matmul(
                            out=st[:, h, :],
                            lhsT=kt_sb[p][hsl, ksl],
                            rhs=qt_sb[p][hsl, qsl],
                            start=True,
                            stop=True,
                        )
                    pt = ptp.tile([128, 2, 512], FP16, tag="pt", name="pt")
                    nc.scalar.activation(out=pt[:], in_=st[:], func=Exp, scale=SCALE)
                    if t == 1 and fin_pending is not None:
                        fin_pending()
                        fin_pending = None
                    if prev is not None:
                        prev()
                        prev = None

                    def step_pv(kb=kb, qb=qb, pt=pt, o_ps=o_ps, t=t, j=j):
                        for h in range(2):
                            nc.tensor.matmul(
                                out=o_ps[h * D : (h + 1) * D, :],
                                lhsT=v_sb[kb][:, 2 * p + h, :],
                                rhs=pt[:, h, :],
                                start=(t == 0),
                                stop=(t == 3),
                                tile_position=(0, h * D),
                                skip_group_check=True,
                            )
                        # evac first: the next cell's o alloc waits on it,
                        # so it must not queue behind the ssum add on DVE
                        if t == 3:
                            if j == 0:
                                nc.vector.tensor_copy(out=o_acc[qb][:], in_=o_ps[:])
                            else:
                                nc.vector.tensor_add(
                                    out=o_acc[qb][:], in0=o_acc[qb][:], in1=o_ps[:]
                                )
                        sj = ssum_sb[qb][kb % 2]
                        if kb < 2:
                            nc.vector.tensor_copy(out=sj[:], in_=pt[:])
                        else:
                            nc.vector.tensor_add(out=sj[:], in0=sj[:], in1=pt[:])

                    prev = step_pv
                    for fn in filler_map.get((qb, kb), ()):
                        fn()

                if j == 3:
                    def strip_fin(qb=qb, qsl=qsl):
                        s_ps = ppsum.tile([33, 512], F32, tag="qkps", name="sps")
                        for h in range(2):
                            for j2 in range(2):
                                nc.tensor.matmul(
                                    out=s_ps[32 * h : 32 * h + 1, :],
                                    lhsT=ones_sb[:],
                                    rhs=ssum_sb[qb][j2][:, h, :],
                                    start=(j2 == 0),
                                    stop=(j2 == 1),
                                    tile_position=(0, 32 * h),
                                    skip_group_check=True,
                                )
                        ss = otp.tile([33, 512], F32, tag="ss", name="ss")
                        for h in range(2):
                            nc.vector.tensor_copy(
                                out=ss[32 * h : 32 * h + 1, :],
                                in_=s_ps[32 * h : 32 * h + 1, :],
                            )
                        ss_view = bass.AP(
                            tensor=ss.tensor, offset=ss.offset,
                            ap=[[32 * ss.ap[0][0], 2]] + list(ss.ap[1:]),
                        )
                        nc.sync.dma_start(out=out_s[p, :, qsl], in_=ss_view)
                        ot = otp.tile([128, 512], BF16, tag="ot", name="ot")
                        nc.vector.tensor_copy(out=ot[:], in_=o_acc[qb][:])
                        nc.sync.dma_start(out=out_o[p, :, qsl], in_=ot[:])

                    fin_pending = strip_fin
            prev()
            return fin_pending

        def attn(p, filler_hook=None, carry_fin=None):
            fin_pending = carry_fin
            for qb in range(QB):
                qsl = slice(qb * 512, (qb + 1) * 512)
                # both heads' O^T col-packed: head h at partitions h*64..
                o_ps = opp.tile([128, 512], F32, tag="o", name="o")
                # running sums of P^T chunks (softmax denominators): two
                # fp16 parity accumulators keep the DVE in its fast 2-byte
                # mode and halve the accumulation depth.
                ssum = [
                    ssp.tile([128, 2, 512], FP16, tag=f"ssum{j}", name=f"ssum{j}")
                    for j in range(2)
                ]

                def emit_pv(args):
                    kb, pt = args
                    for h in range(2):
                        nc.tensor.matmul(
                            out=o_ps[h * D : (h + 1) * D, :],
                            lhsT=v_sb[kb][:, 2 * p + h, :],
                            rhs=pt[:, h, :],
                            start=(kb == 0),
                            stop=(kb == KB - 1),
                            tile_position=(0, h * D),
                            skip_group_check=True,
                        )
                    sj = ssum[kb % 2]
                    if kb < 2:
                        nc.vector.tensor_copy(out=sj[:], in_=pt[:])
                    else:
                        nc.vector.tensor_add(out=sj[:], in0=sj[:], in1=pt[:])

                # Per step: QK -> exp -> PV(prev) -> fillers.  The QK/exp
                # chain leads; PV lags one step (pt pool decouples); filler
                # projection blocks absorb the PE slack under the ACT-bound
                # exp stream.
                prev = None
                for kb in range(KB):
                    ksl = slice(kb * 128, (kb + 1) * 128)
                    # st layout [128 keys, head, 512 q] fp32: head h
                    # occupies its own PSUM bank; ring-3 lets QK run ~2
                    # steps ahead of the exp stream.
                    st = stp.tile([128, 2, 512], F32, tag="st", name="st")
                    for h in range(2):
                        hsl = slice(h * D, (h + 1) * D)
                        nc.tensor.matmul(
                            out=st[:, h, :],
                            lhsT=kt_sb[p][hsl, ksl],
                            rhs=qt_sb[p][hsl, qsl],
                            start=True,
                            stop=True,
                        )
                    pt = ptp.tile([128, 2, 512], FP16, tag="pt", name="pt")
                    nc.scalar.activation(out=pt[:], in_=st[:], func=Exp, scale=SCALE)
                    if kb == 1 and fin_pending is not None:
                        fin_pending()
                        fin_pending = None
                    if prev is not None:
                        emit_pv(prev)
                    prev = (kb, pt)
                    if filler_hook is not None:
                        filler_hook(qb, kb)
                emit_pv(prev)

                # Finalize (partition-reduce the running sums with
                # ones-vector matmuls -- both parity accumulators accumulate
                # into the same PSUM row, head h at PSUM partition 32*h --
                # then evacuate sums + O and DMA out).  Deferred into the
                # next qb's step 1 so it never sits ahead of the next qb's
                # QK chain in the engine FIFOs.
                def finalize(qb=qb, qsl=qsl, o_ps=o_ps, ssum=ssum):
                    s_ps = ppsum.tile([33, 512], F32, tag="qkps", name="sps")
                    for h in range(2):
                        for j in range(2):
                            nc.tensor.matmul(
                                out=s_ps[32 * h : 32 * h + 1, :],
                                lhsT=ones_sb[:],
                                rhs=ssum[j][:, h, :],
                                start=(j == 0),
                                stop=(j == 1),
                                tile_position=(0, 32 * h),
                                skip_group_check=True,
                            )
                    ss = otp.tile([33, 512], F32, tag="ss", name="ss")
                    for h in range(2):
                        nc.vector.tensor_copy(
                            out=ss[32 * h : 32 * h + 1, :],
                            in_=s_ps[32 * h : 32 * h + 1, :],
                        )
                    ss_view = bass.AP(
                        tensor=ss.tensor, offset=ss.offset,
                        ap=[[32 * ss.ap[0][0], 2]] + list(ss.ap[1:]),
                    )
                    nc.sync.dma_start(out=out_s[p, :, qsl], in_=ss_view)
                    ot = otp.tile([128, 512], BF16, tag="ot", name="ot")
                    nc.vector.tensor_copy(out=ot[:], in_=o_ps[:])
                    nc.sync.dma_start(out=out_o[p, :, qsl], in_=ot[:])

                fin_pending = finalize
            return fin_pending

        def proj_qk_first():
            qps = ppsum.tile([128, 512], F32, tag="qkps", name="qkps")
            kps = ppsum.tile([128, 512], F32, tag="qkps", name="qkps")
            for k in range(KC):
                for w_m, ps in ((wq_m, qps), (wk_m, kps)):
                    nc.tensor.matmul(
                        out=ps[:],
                        lhsT=w_m[:, 0, k, :],
                        rhs=xt_sb[k][:, 0:512],
                        start=(k == 0),
                        stop=(k == KC - 1),
                    )
            nc.vector.tensor_scalar_add(out=qt_sb[0][:, 0:512], in0=qps[:], scalar1=bq_sb[0][:])
            nc.vector.tensor_scalar_add(out=kt_sb[0][:, 0:512], in0=kps[:], scalar1=bk_sb[0][:])

        # Filler schedule.  Pair-0 (strip mode) keys fillers by the actual
        # (qb, kb) of each step; blocks are placed after their input DMA
        # lands (nb1 ~30us, nb2 ~38, nb3 ~46, wv-m1 ~52) and at least one
        # cell before their consumer.
        def sched(table, qb, kb):
            for (q, s), fn in table:
                if q == qb and s == kb:
                    fn()

        def V0(kb):
            return lambda: proj_v_block(kb, 0)

        def V1(kb):
            return lambda: proj_v_block(kb, 1)

        def PQ(which, m, nb, part):
            return lambda: proj_qk_half(which, m, nb, part)

        p0_map = {
            (0, 0): [V0(1)], (0, 1): [V0(2)],
            (0, 2): [PQ("k", 0, 1, 0)], (0, 3): [PQ("k", 0, 1, 1), V0(3)],
            (0, 4): [PQ("q", 0, 1, 0), V0(4)], (0, 5): [PQ("q", 0, 1, 1), V0(5)],
            (0, 6): [V0(6)], (0, 7): [V0(7)],
            (1, 4): [PQ("k", 0, 2, 0)], (1, 5): [PQ("k", 0, 2, 1)],
            (0, 8): [V0(8)], (0, 9): [V0(9)], (0, 10): [V0(10)], (0, 11): [V0(11)],
            (1, 8): [PQ("q", 0, 2, 0)], (1, 9): [PQ("q", 0, 2, 1)],
            (2, 0): [PQ("q", 0, 3, 0)], (2, 1): [PQ("q", 0, 3, 1)],
            (2, 4): [V1(0)], (2, 5): [PQ("k", 0, 3, 0)],
            (2, 6): [V1(1)], (2, 7): [PQ("k", 0, 3, 1)],
            (2, 8): [V1(2)], (2, 10): [V1(3)],
            (0, 12): [V0(12)], (0, 13): [V0(13)], (0, 14): [V0(14)], (0, 15): [V0(15)],
            (3, 0): [V1(4)], (3, 1): [V1(5)],
            (3, 2): [PQ("k", 1, 0, 0)], (3, 3): [PQ("k", 1, 0, 1)],
            (1, 12): [V1(6)], (1, 13): [V1(7)], (1, 14): [V1(8)], (1, 15): [V1(9)],
            (3, 4): [V1(10)], (3, 7): [V1(11)],
            (3, 5): [PQ("q", 1, 0, 0)], (3, 6): [PQ("q", 1, 0, 1)],
            (2, 12): [PQ("k", 1, 1, 0)], (2, 13): [PQ("k", 1, 1, 1)],
            (2, 14): [V1(12)], (2, 15): [V1(13)],
            (3, 8): [V1(14)], (3, 9): [V1(15)],
        }

        p1_table = [
            ((0, 2), lambda: proj_qk_half("k", 1, 2, 0)),
            ((0, 3), lambda: proj_qk_half("k", 1, 2, 1)),
            ((0, 6), lambda: proj_qk_half("k", 1, 3, 0)),
            ((0, 7), lambda: proj_qk_half("k", 1, 3, 1)),
            ((0, 10), lambda: proj_qk_half("q", 1, 1, 0)),
            ((0, 11), lambda: proj_qk_half("q", 1, 1, 1)),
            ((1, 1), lambda: proj_qk_half("q", 1, 2, 0)),
            ((1, 2), lambda: proj_qk_half("q", 1, 2, 1)),
            ((1, 7), lambda: proj_qk_half("q", 1, 3, 0)),
            ((1, 8), lambda: proj_qk_half("q", 1, 3, 1)),
        ]

        proj_qk_first()
        proj_v_block(0, 0)
        fin = attn_strip(0, p0_map)
        fin = attn(1, filler_hook=lambda qb, kb: sched(p1_table, qb, kb),
                   carry_fin=fin)
        fin()


def build_nc():
    nc = bacc.Bacc(
        "TRN2",
        target_bir_lowering=False,
        debug=False,
        num_devices=NCORES,
        enable_partition_id=False,
    )
    xt = nc.dram_tensor("xt", [C, N], BF16, kind="ExternalInput").ap()
    wqt = nc.dram_tensor("wqt", [128, 2 + 2048], BF16, kind="ExternalInput").ap()
    wkt = nc.dram_tensor("wkt", [128, 2 + 2048], BF16, kind="ExternalInput").ap()
    wvt = nc.dram_tensor("wvt", [128, 2048], BF16, kind="ExternalInput").ap()
    out_o = nc.dram_tensor("out_o", [2, 128, N], BF16, kind="ExternalOutput").ap()
    out_s = nc.dram_tensor("out_s", [2, 2, N], F32, kind="ExternalOutput").ap()

    with tile.TileContext(nc) as tc:
        build_kernel(tc, xt, wqt, wkt, wvt, out_o, out_s)
    nc.compile()
    return nc


def _w_prep(w, sl, bias=None):
    # [HD-slice, C] weight -> SBUF-ready m-major [128, (2 kd)]: element
    # (c=k*128+p, h=m*128+j) -> [p, m, k, j], flattened; with the two bias
    # columns (bias[m*128+p] on partition p) prepended when given.
    wt = np.asarray(w, np.float32)[sl, :].T  # [C, HD]
    wt = wt.reshape(KC, 128, 2, 128).transpose(1, 2, 0, 3).reshape(128, 2048)
    if bias is None:
        return np.ascontiguousarray(wt).astype(ml_dtypes.bfloat16)
    b = np.asarray(bias, np.float32)[sl].reshape(2, 128).T  # [128, 2]
    return np.ascontiguousarray(np.concatenate([b, wt], axis=1)).astype(
        ml_dtypes.bfloat16
    )


def shard_inputs(inputs):
    x = np.asarray(inputs["x"], np.float32)
    in_maps = []
    for core in range(NCORES):
        b, g = core // 4, core % 4
        sl = slice(g * HD, (g + 1) * HD)
        in_maps.append(
            {
                "xt": np.ascontiguousarray(x[b].T).astype(ml_dtypes.bfloat16),
                "wqt": _w_prep(inputs["Wq"], sl, inputs["bq"]),
                "wkt": _w_prep(inputs["Wk"], sl, inputs["bk"]),
                "wvt": _w_prep(inputs["Wv"], sl),
            }
        )
    return in_maps


def assemble(results, inputs, B=2):
    bv = np.asarray(inputs["bv"], np.float32)
    out = np.zeros((B, N, C), np.float32)
    for core in range(NCORES):
        b, g = core // 4, core % 4
        oo = np.asarray(results[core]["out_o"], np.float32)  # [2, 128, N]
        os_ = np.asarray(results[core]["out_s"], np.float32)  # [2, 2, N]
        o = oo.reshape(2, 2, D, N)  # [pair, head, d, n]
        on = o / os_[:, :, None, :]
        # [pair, head, d, n] -> [n, pair*2*D + head*D + d], + host-side bv
        out[b, :, g * HD : (g + 1) * HD] = (
            on.transpose(3, 0, 1, 2).reshape(N, HD) + bv[g * HD : (g + 1) * HD]
        )
    return out


_NC_CACHE = None


def _get_nc():
    global _NC_CACHE
    if _NC_CACHE is None:
        _NC_CACHE = build_nc()
    return _NC_CACHE


def kernel(**inputs):
    nc = _get_nc()
    in_maps = shard_inputs(inputs)
    res = run_bass_kernel_spmd(
        nc,
        in_maps,
        core_ids=list(range(NCORES)),
        trace=bool(int(os.environ.get("KERNEL_TRACE", "0"))),
    )
    return assemble(res.results, inputs, B=int(np.asarray(inputs["x"]).shape[0]))


# revision 47
# speedup vs baseline: 1.2006x; 1.0265x over previous
"""Multi-head attention forward kernel for Trainium2 (8 NeuronCores).

Problem: B=2, N=2048, C=1024, H=16 heads, head_dim=64.
    q = x @ Wq.T + bq  (same for k, v)
    out = softmax(q k^T / sqrt(C)) v       (per head), re-merged to [B, N, C]

Sharding: core = (batch b, head-group g): b = core // 4, g = core % 4.
Each core computes 4 heads of one batch element. No collectives needed --
outputs are disjoint; host gathers and finishes with a cheap epilogue
(normalize by the row-sums, add the V bias, transpose).

v2 design notes (measured atoms from microbench):
  - Any 512-col MM "slot" (single, row-packed pair, col-packed pair) paces
    at ~259 ns back-to-back; LDWEIGHTS hides completely. PE total ~125 us.
  - ACT exp from PSUM runs ~1.18 ns/elem regardless of op size ->
    ACT busy floor ~155 us/core. ACT is THE bottleneck; everything else
    is scheduled to keep the exp stream gapless.
  - st ring-3 (stp bufs=3, 6 PSUM banks) so QK can run 2 steps ahead of
    exp; o_ps 1 bank; proj+sums share 1 bank (ppsum bufs=1).
  - V bias is softmax-invariant additive on the output -> applied on host;
    V evacuates via plain tensor_copy (cheaper DVE).
  - out_o shipped as bf16 (halves out-DMA, 2x DVE copy mode).
  - Granular input DMA (w's first, then xt in nb-major 512-col slices) so
    the first exp fires ~8 us in; V blocks 0-2 emitted in the prologue,
    V[s+2] per qb0 step s, kt/qt projection blocks as PE filler inside the
    ACT-bound window (emission order = scheduler priority).
Outputs: out_o [2, 128, N] bf16 (pair, head-major O^T rows, queries),
         out_s [2, 2, N] f32   (pair, head, query sums).
"""

import os
import sys

import ml_dtypes
import numpy as np

for _p in ("/opt/trn_rl_repo",):
    if _p not in sys.path:
        sys.path.insert(0, _p)

import concourse.bass as bass  # noqa: E402
import concourse.tile as tile  # noqa: E402
from concourse import bacc, mybir  # noqa: E402
from concourse.bass_utils import run_bass_kernel_spmd  # noqa: E402

N = 2048  # sequence length
C = 1024  # model dim
D = 64  # head dim
NH = 4  # heads per core
HD = NH * D  # 256 output channels per core
NCORES = 8
KB = N // 128  # 16 key chunks of 128
QB = N // 512  # 4 query blocks of 512
KC = C // 128  # 8 contraction chunks for projections
SCALE = 1.0 / 32.0  # 1 / sqrt(C)

F32 = mybir.dt.float32
BF16 = mybir.dt.bfloat16
FP16 = mybir.dt.float16


def build_kernel(tc, xt, wqt, wkt, wvt, out_o, out_s):
    nc = tc.nc
    Exp = mybir.ActivationFunctionType.Exp

    with (
        tc.tile_pool(name="res", bufs=1) as res,
        tc.tile_pool(name="ppsum", bufs=1, space="PSUM") as ppsum,
        tc.tile_pool(name="stp", bufs=3, space="PSUM") as stp,
        tc.tile_pool(name="opp", bufs=1, space="PSUM") as opp,
        tc.tile_pool(name="ptp", bufs=10) as ptp,
        tc.tile_pool(name="otp", bufs=2) as otp,
        tc.tile_pool(name="ssp", bufs=2) as ssp,
    ):
        # ---- resident SBUF tensors ----
        # W layout [128, 2 bias cols + (m, k, d) m-major weights]: the two
        # bias columns ride inside the same contiguous DMA (a standalone
        # [128,1] bias DMA is a 4-byte-packet storm that stalls the queue),
        # and the m-major order lets each head-pair half load separately.
        wq_flat = res.tile([128, 2 + 2048], BF16, tag="wq", name="wq")
        wk_flat = res.tile([128, 2 + 2048], BF16, tag="wk", name="wk")
        wv_flat = res.tile([128, 2048], BF16, tag="wv", name="wv")
        xt_all = res.tile([128, KC, N], BF16, tag="xt", name="xt")
        xt_sb = [xt_all[:, k, :] for k in range(KC)]
        wq_m = wq_flat[:, 2:].rearrange("p (m k d) -> p m k d", m=2, k=KC)
        wk_m = wk_flat[:, 2:].rearrange("p (m k d) -> p m k d", m=2, k=KC)
        wv_m = wv_flat.rearrange("p (m k d) -> p m k d", m=2, k=KC)
        bqf = res.tile([128, 2], F32, tag="bqf", name="bqf")
        bkf = res.tile([128, 2], F32, tag="bkf", name="bkf")
        bq_sb = [bqf[:, m : m + 1] for m in range(2)]
        bk_sb = [bkf[:, m : m + 1] for m in range(2)]
        qt_sb = [res.tile([128, N], BF16, tag=f"qt{m}", name=f"qt{m}") for m in range(2)]
        kt_sb = [res.tile([128, N], BF16, tag=f"kt{m}", name=f"kt{m}") for m in range(2)]
        v_sb = [res.tile([128, NH, D], FP16, tag=f"v{kb}", name=f"v{kb}") for kb in range(KB)]
        ones_sb = res.tile([128, 1], FP16, tag="ones", name="ones")
        warm_sb = res.tile([1, 2], F32, tag="warm", name="warm")
        # pair-0 strip-mode state: resident softmax-sum parity accumulators
        # (all 4 qbs live at once) and per-qb SBUF O accumulators that
        # collect 4-kb strip partials from the single o PSUM bank.
        ssum_sb = [
            [res.tile([128, 2, 512], FP16, tag=f"ss{q}{j}", name=f"ss{q}{j}")
             for j in range(2)]
            for q in range(QB)
        ]
        o_acc = [res.tile([128, 512], F32, tag=f"oa{q}", name=f"oa{q}")
                 for q in range(QB)]

        # ---- input DMAs, ordered by consumer deadline (HBM bandwidth is
        # shared by all 8 cores; the whole input set takes tens of us).
        # FEW, BIG descriptors: each DMA_DIRECT2D trigger costs ~600 ns on
        # the sync queue and >.30 queued descriptors stall on ring space,
        # delaying later transfers by ~10 us (and the resulting PE idle
        # re-throttles HAM to K=4/8).  Partition lines stay >=1KB. ----
        # Two hardware DGE queues (SP + Activation) run in parallel: the
        # critical xt nb0/nb1 stream goes on the scalar queue (idle until
        # the first exp anyway) while the W stream runs on sync, halving
        # the serial prefix before the first exp.
        xtr = xt.rearrange("(k p) n -> p k n", p=128)
        nc.scalar.dma_start(out=xt_all[:, 0:4, 0:512], in_=xtr[:, 0:4, 0:512])
        nc.scalar.dma_start(out=xt_all[:, 4:8, 0:512], in_=xtr[:, 4:8, 0:512])
        nc.scalar.dma_start(out=xt_all[:, :, 512:1024], in_=xtr[:, :, 512:1024])
        for half in range(2):
            lo, hi = 2 + half * 512, 2 + (half + 1) * 512
            nc.sync.dma_start(out=wq_flat[:, (0 if half == 0 else lo) : hi],
                              in_=wqt[:, (0 if half == 0 else lo) : hi])
            nc.sync.dma_start(out=wk_flat[:, (0 if half == 0 else lo) : hi],
                              in_=wkt[:, (0 if half == 0 else lo) : hi])
        nc.sync.dma_start(out=wv_flat[:, 0:1024], in_=wvt[:, 0:1024])
        for nb in range(2, QB):
            nsl = slice(nb * 512, (nb + 1) * 512)
            nc.sync.dma_start(out=xt_all[:, :, nsl], in_=xtr[:, :, nsl])
        # pair-1 halves: needed only from pair0-qb2 onwards
        nc.sync.dma_start(out=wq_flat[:, 2 + 1024 :], in_=wqt[:, 2 + 1024 :])
        nc.sync.dma_start(out=wk_flat[:, 2 + 1024 :], in_=wkt[:, 2 + 1024 :])
        nc.sync.dma_start(out=wv_flat[:, 1024:], in_=wvt[:, 1024:])
        nc.vector.memset(ones_sb[:], 1.0)
        # widen the in-DMA bf16 bias columns to f32 for tensor_scalar
        nc.vector.tensor_copy(out=bqf[:], in_=wq_flat[:, 0:2])
        nc.vector.tensor_copy(out=bkf[:], in_=wk_flat[:, 0:2])
        # warm up the ACT exp table while DMAs land
        nc.vector.memset(warm_sb[:], 0.0)
        nc.scalar.activation(out=warm_sb[:, 0:1], in_=warm_sb[:, 1:2], func=Exp)

        # Projection blocks are emitted in HALVES (4 contraction chunks
        # each) on consecutive steps so a block never overflows a single
        # ACT-bound step window and stalls the exp chain through the PE
        # FIFO.  The live PSUM tile is kept in `pending` between halves.
        pending = {}

        def proj_qk_half(which, m, nb, part):
            key = (which, m, nb)
            nsl = slice(nb * 512, (nb + 1) * 512)
            if part == 0:
                pending[key] = ppsum.tile([128, 512], F32, tag="qkps", name="qkps")
            ps = pending[key]
            w_m = wq_m if which == "q" else wk_m
            for k in range(part * 4, part * 4 + 4):
                nc.tensor.matmul(
                    out=ps[:],
                    lhsT=w_m[:, m, k, :],
                    rhs=xt_sb[k][:, nsl],
                    start=(k == 0),
                    stop=(k == KC - 1),
                )
            if part == 1:
                b_sb = (bq_sb if which == "q" else bk_sb)[m]
                t_sb = (qt_sb if which == "q" else kt_sb)[m]
                nc.vector.tensor_scalar_add(out=t_sb[:, nsl], in0=ps[:], scalar1=b_sb[:])
                del pending[key]

        def proj_qk_block(which, m, nb):
            proj_qk_half(which, m, nb, 0)
            proj_qk_half(which, m, nb, 1)

        def proj_v_block(kb, half):
            # one head-pair's V columns: pair-0's V is needed from the very
            # first PV steps, pair-1's only once pair 1 starts -> split so
            # qb0 carries half the V-projection load.
            vps = ppsum.tile([128, 128], F32, tag="qkps", name="vps")
            for k in range(KC):
                nc.tensor.matmul(
                    out=vps[:],
                    lhsT=xt_sb[k][:, kb * 128 : (kb + 1) * 128],
                    rhs=wv_m[:, half, k, :],
                    start=(k == 0),
                    stop=(k == KC - 1),
                )
            # V bias is applied on the host (softmax-invariant): plain copy.
            nc.vector.tensor_copy(
                out=v_sb[kb][:, 2 * half : 2 * half + 2, :],
                in_=vps[:].rearrange("p (h d) -> p h d", h=2),
            )

        def attn_strip(p, filler_map):
            """Pair-0 attention in 4-kb-strip x qb cells, ordered so early
            cells only consume xt nb0/nb1 -- the later x slices stream in
            behind the compute instead of stalling the exp chain."""
            cells = [(0, 0), (1, 0), (0, 1), (1, 1), (2, 0), (2, 1), (0, 2),
                     (1, 2), (2, 2), (3, 0), (0, 3), (3, 1), (1, 3), (3, 2),
                     (2, 3), (3, 3)]
            prev = None       # pending PV(+evac) closure from the last step
            fin_pending = None
            for (j, i) in cells:
                qb = i
                qsl = slice(qb * 512, (qb + 1) * 512)
                o_ps = opp.tile([128, 512], F32, tag="o", name="o")
                for t in range(4):
                    kb = 4 * j + t
                    ksl = slice(kb * 128, (kb + 1) * 128)
                    st = stp.tile([128, 2, 512], F32, tag="st", name="st")
                    for h in range(2):
                        hsl = slice(h * D, (h + 1) * D)
                        nc.tensor.matmul(
                            out=st[:, h, :],
                            lhsT=kt_sb[p][hsl, ksl],
                            rhs=qt_sb[p][hsl, qsl],
                            start=True,
                            stop=True,
                        )
                    pt = ptp.tile([128, 2, 512], FP16, tag="pt", name="pt")
                    nc.scalar.activation(out=pt[:], in_=st[:], func=Exp, scale=SCALE)
                    if t == 1 and fin_pending is not None:
                        fin_pending()
                        fin_pending = None
                    if prev is not None:
                        prev()
                        prev = None

                    def step_pv(kb=kb, qb=qb, pt=pt, o_ps=o_ps, t=t, j=j):
                        for h in range(2):
                            nc.tensor.matmul(
                                out=o_ps[h * D : (h + 1) * D, :],
                                lhsT=v_sb[kb][:, 2 * p + h, :],
                                rhs=pt[:, h, :],
                                start=(t == 0),
                                stop=(t == 3),
                                tile_position=(0, h * D),
                                skip_group_check=True,
                            )
                        # evac first: the next cell's o alloc waits on it,
                        # so it must not queue behind the ssum add on DVE
                        if t == 3:
                            if j == 0:
                                nc.vector.tensor_copy(out=o_acc[qb][:], in_=o_ps[:])
                            else:
                                nc.vector.tensor_add(
                                    out=o_acc[qb][:], in0=o_acc[qb][:], in1=o_ps[:]
                                )
                        sj = ssum_sb[qb][kb % 2]
                        if kb < 2:
                            nc.vector.tensor_copy(out=sj[:], in_=pt[:])
                        else:
                            nc.vector.tensor_add(out=sj[:], in0=sj[:], in1=pt[:])

                    prev = step_pv
                    for fn in filler_map.get((qb, kb), ()):
                        fn()

                if j == 3:
                    def strip_fin(qb=qb, qsl=qsl):
                        s_ps = ppsum.tile([33, 512], F32, tag="qkps", name="sps")
                        for h in range(2):
                            for j2 in range(2):
                                nc.tensor.matmul(
                                    out=s_ps[32 * h : 32 * h + 1, :],
                                    lhsT=ones_sb[:],
                                    rhs=ssum_sb[qb][j2][:, h, :],
                                    start=(j2 == 0),
                                    stop=(j2 == 1),
                                    tile_position=(0, 32 * h),
                                    skip_group_check=True,
                                )
                        ss = otp.tile([33, 512], F32, tag="ss", name="ss")
                        for h in range(2):
                            nc.vector.tensor_copy(
                                out=ss[32 * h : 32 * h + 1, :],
                                in_=s_ps[32 * h : 32 * h + 1, :],
                            )
                        ss_view = bass.AP(
                            tensor=ss.tensor, offset=ss.offset,
                            ap=[[32 * ss.ap[0][0], 2]] + list(ss.ap[1:]),
                        )
                        nc.sync.dma_start(out=out_s[p, :, qsl], in_=ss_view)
                        ot = otp.tile([128, 512], BF16, tag="ot", name="ot")
                        nc.vector.tensor_copy(out=ot[:], in_=o_acc[qb][:])
                        nc.sync.dma_start(out=out_o[p, :, qsl], in_=ot[:])

                    fin_pending = strip_fin
            prev()
            return fin_pending

        def attn(p, filler_hook=None, carry_fin=None):
            fin_pending = carry_fin
            for qb in range(QB):
                qsl = slice(qb * 512, (qb + 1) * 512)
                # both heads' O^T col-packed: head h at partitions h*64..
                o_ps = opp.tile([128, 512], F32, tag="o", name="o")
                # running sums of P^T chunks (softmax denominators): two
                # fp16 parity accumulators keep the DVE in its fast 2-byte
                # mode and halve the accumulation depth.
                ssum = [
                    ssp.tile([128, 2, 512], FP16, tag=f"ssum{j}", name=f"ssum{j}")
                    for j in range(2)
                ]

                def emit_pv(args):
                    kb, pt = args
                    for h in range(2):
                        nc.tensor.matmul(
                            out=o_ps[h * D : (h + 1) * D, :],
                            lhsT=v_sb[kb][:, 2 * p + h, :],
                            rhs=pt[:, h, :],
                            start=(kb == 0),
                            stop=(kb == KB - 1),
                            tile_position=(0, h * D),
                            skip_group_check=True,
                        )
                    sj = ssum[kb % 2]
                    if kb < 2:
                        nc.vector.tensor_copy(out=sj[:], in_=pt[:])
                    else:
                        nc.vector.tensor_add(out=sj[:], in0=sj[:], in1=pt[:])

                # Per step: QK -> exp -> PV(prev) -> fillers.  The QK/exp
                # chain leads; PV lags one step (pt pool decouples); filler
                # projection blocks absorb the PE slack under the ACT-bound
                # exp stream.
                prev = None
                for kb in range(KB):
                    ksl = slice(kb * 128, (kb + 1) * 128)
                    # st layout [128 keys, head, 512 q] fp32: head h
                    # occupies its own PSUM bank; ring-3 lets QK run ~2
                    # steps ahead of the exp stream.
                    st = stp.tile([128, 2, 512], F32, tag="st", name="st")
                    for h in range(2):
                        hsl = slice(h * D, (h + 1) * D)
                        nc.tensor.matmul(
                            out=st[:, h, :],
                            lhsT=kt_sb[p][hsl, ksl],
                            rhs=qt_sb[p][hsl, qsl],
                            start=True,
                            stop=True,
                        )
                    pt = ptp.tile([128, 2, 512], FP16, tag="pt", name="pt")
                    nc.scalar.activation(out=pt[:], in_=st[:], func=Exp, scale=SCALE)
                    if kb == 1 and fin_pending is not None:
                        fin_pending()
                        fin_pending = None
                    if prev is not None:
                        emit_pv(prev)
                    prev = (kb, pt)
                    if filler_hook is not None:
                        filler_hook(qb, kb)
                emit_pv(prev)

                # Finalize (partition-reduce the running sums with
                # ones-vector matmuls -- both parity accumulators accumulate
                # into the same PSUM row, head h at PSUM partition 32*h --
                # then evacuate sums + O and DMA out).  Deferred into the
                # next qb's step 1 so it never sits ahead of the next qb's
                # QK chain in the engine FIFOs.
                def finalize(qb=qb, qsl=qsl, o_ps=o_ps, ssum=ssum):
                    s_ps = ppsum.tile([33, 512], F32, tag="qkps", name="sps")
                    for h in range(2):
                        for j in range(2):
                            nc.tensor.matmul(
                                out=s_ps[32 * h : 32 * h + 1, :],
                                lhsT=ones_sb[:],
                                rhs=ssum[j][:, h, :],
                                start=(j == 0),
                                stop=(j == 1),
                                tile_position=(0, 32 * h),
                                skip_group_check=True,
                            )
                    ss = otp.tile([33, 512], F32, tag="ss", name="ss")
                    for h in range(2):
                        nc.vector.tensor_copy(
                            out=ss[32 * h : 32 * h + 1, :],
                            in_=s_ps[32 * h : 32 * h + 1, :],
                        )
                    ss_view = bass.AP(
                        tensor=ss.tensor, offset=ss.offset,
                        ap=[[32 * ss.ap[0][0], 2]] + list(ss.ap[1:]),
                    )
                    nc.sync.dma_start(out=out_s[p, :, qsl], in_=ss_view)
                    ot = otp.tile([128, 512], BF16, tag="ot", name="ot")
                    nc.vector.tensor_copy(out=ot[:], in_=o_ps[:])
                    nc.sync.dma_start(out=out_o[p, :, qsl], in_=ot[:])

                fin_pending = finalize
            return fin_pending

        def proj_qk_first():
            qps = ppsum.tile([128, 512], F32, tag="qkps", name="qkps")
            kps = ppsum.tile([128, 512], F32, tag="qkps", name="qkps")
            for k in range(KC):
                for w_m, ps in ((wq_m, qps), (wk_m, kps)):
                    nc.tensor.matmul(
                        out=ps[:],
                        lhsT=w_m[:, 0, k, :],
                        rhs=xt_sb[k][:, 0:512],
                        start=(k == 0),
                        stop=(k == KC - 1),
                    )
            nc.vector.tensor_scalar_add(out=qt_sb[0][:, 0:512], in0=qps[:], scalar1=bq_sb[0][:])
            nc.vector.tensor_scalar_add(out=kt_sb[0][:, 0:512], in0=kps[:], scalar1=bk_sb[0][:])

        # Filler schedule.  Pair-0 (strip mode) keys fillers by the actual
        # (qb, kb) of each step; blocks are placed after their input DMA
        # lands (nb1 ~30us, nb2 ~38, nb3 ~46, wv-m1 ~52) and at least one
        # cell before their consumer.
        def sched(table, qb, kb):
            for (q, s), fn in table:
                if q == qb and s == kb:
                    fn()

        def V0(kb):
            return lambda: proj_v_block(kb, 0)

        def V1(kb):
            return lambda: proj_v_block(kb, 1)

        def PQ(which, m, nb, part):
            return lambda: proj_qk_half(which, m, nb, part)

        p0_map = {
            (0, 0): [V0(1)], (0, 1): [V0(2)],
            # kt-nb1 halves both on the cell's LAST slot: they stall on the
            # nb1 DMA, and anywhere earlier they block the cell's own QKs
            # through the PE FIFO.
            (0, 3): [PQ("k", 0, 1, 0), PQ("k", 0, 1, 1), V0(3)],
            (0, 4): [PQ("q", 0, 1, 0), V0(4)], (0, 5): [PQ("q", 0, 1, 1), V0(5)],
            (0, 6): [V0(6)], (0, 7): [V0(7)],
            (1, 2): [PQ("k", 0, 2, 0)], (1, 3): [PQ("k", 0, 2, 1)],
            (0, 8): [V0(8)], (0, 9): [V0(9)], (0, 10): [V0(10)], (0, 11): [V0(11)],
            (1, 6): [PQ("q", 0, 2, 0)], (1, 7): [PQ("q", 0, 2, 1)],
            (2, 0): [PQ("q", 0, 3, 0)], (2, 1): [PQ("q", 0, 3, 1)],
            (2, 4): [V1(0)], (2, 5): [PQ("k", 0, 3, 0)],
            (2, 6): [V1(1)], (2, 7): [PQ("k", 0, 3, 1)],
            (2, 8): [V1(2)], (2, 10): [V1(3)],
            (0, 12): [V0(12)], (0, 13): [V0(13)], (0, 14): [V0(14)], (0, 15): [V0(15)],
            (3, 0): [V1(4)], (3, 1): [V1(5)],
            (3, 2): [PQ("k", 1, 0, 0)], (3, 3): [PQ("k", 1, 0, 1)],
            (1, 12): [V1(6)], (1, 13): [V1(7)],
            (3, 10): [V1(8)], (3, 11): [V1(9)],
            (3, 4): [V1(10)], (3, 7): [V1(11)],
            (3, 5): [PQ("q", 1, 0, 0)], (3, 6): [PQ("q", 1, 0, 1)],
            (2, 12): [PQ("k", 1, 1, 0)], (2, 13): [PQ("k", 1, 1, 1)],
            (2, 14): [V1(12)], (2, 15): [V1(13)],
            (3, 8): [V1(14)], (3, 9): [V1(15)],
        }

        p1_table = [
            ((0, 2), lambda: proj_qk_half("k", 1, 2, 0)),
            ((0, 3), lambda: proj_qk_half("k", 1, 2, 1)),
            ((0, 6), lambda: proj_qk_half("k", 1, 3, 0)),
            ((0, 7), lambda: proj_qk_half("k", 1, 3, 1)),
            ((0, 10), lambda: proj_qk_half("q", 1, 1, 0)),
            ((0, 11), lambda: proj_qk_half("q", 1, 1, 1)),
            ((1, 1), lambda: proj_qk_half("q", 1, 2, 0)),
            ((1, 2), lambda: proj_qk_half("q", 1, 2, 1)),
            ((1, 7), lambda: proj_qk_half("q", 1, 3, 0)),
            ((1, 8), lambda: proj_qk_half("q", 1, 3, 1)),
        ]

        proj_qk_first()
        proj_v_block(0, 0)
        fin = attn_strip(0, p0_map)
        fin = attn(1, filler_hook=lambda qb, kb: sched(p1_table, qb, kb),
                   carry_fin=fin)
        fin()


def build_nc():
    nc = bacc.Bacc(
        "TRN2",
        target_bir_lowering=False,
        debug=False,
        num_devices=NCORES,
        enable_partition_id=False,
    )
    xt = nc.dram_tensor("xt", [C, N], BF16, kind="ExternalInput").ap()
    wqt = nc.dram_tensor("wqt", [128, 2 + 2048], BF16, kind="ExternalInput").ap()
    wkt = nc.dram_tensor("wkt", [128, 2 + 2048], BF16, kind="ExternalInput").ap()
    wvt = nc.dram_tensor("wvt", [128, 2048], BF16, kind="ExternalInput").ap()
    out_o = nc.dram_tensor("out_o", [2, 128, N], BF16, kind="ExternalOutput").ap()
    out_s = nc.dram_tensor("out_s", [2, 2, N], F32, kind="ExternalOutput").ap()

    with tile.TileContext(nc) as tc:
        build_kernel(tc, xt, wqt, wkt, wvt, out_o, out_s)
    nc.compile()
    return nc


def _w_prep(w, sl, bias=None):
    # [HD-slice, C] weight -> SBUF-ready m-major [128, (2 kd)]: element
    # (c=k*128+p, h=m*128+j) -> [p, m, k, j], flattened; with the two bias
    # columns (bias[m*128+p] on partition p) prepended when given.
    wt = np.asarray(w, np.float32)[sl, :].T  # [C, HD]
    wt = wt.reshape(KC, 128, 2, 128).transpose(1, 2, 0, 3).reshape(128, 2048)
    if bias is None:
        return np.ascontiguousarray(wt).astype(ml_dtypes.bfloat16)
    b = np.asarray(bias, np.float32)[sl].reshape(2, 128).T  # [128, 2]
    return np.ascontiguousarray(np.concatenate([b, wt], axis=1)).astype(
        ml_dtypes.bfloat16
    )


def shard_inputs(inputs):
    x = np.asarray(inputs["x"], np.float32)
    in_maps = []
    for core in range(NCORES):
        b, g = core // 4, core % 4
        sl = slice(g * HD, (g + 1) * HD)
        in_maps.append(
            {
                "xt": np.ascontiguousarray(x[b].T).astype(ml_dtypes.bfloat16),
                "wqt": _w_prep(inputs["Wq"], sl, inputs["bq"]),
                "wkt": _w_prep(inputs["Wk"], sl, inputs["bk"]),
                "wvt": _w_prep(inputs["Wv"], sl),
            }
        )
    return in_maps


def assemble(results, inputs, B=2):
    bv = np.asarray(inputs["bv"], np.float32)
    out = np.zeros((B, N, C), np.float32)
    for core in range(NCORES):
        b, g = core // 4, core % 4
        oo = np.asarray(results[core]["out_o"], np.float32)  # [2, 128, N]
        os_ = np.asarray(results[core]["out_s"], np.float32)  # [2, 2, N]
        o = oo.reshape(2, 2, D, N)  # [pair, head, d, n]
        on = o / os_[:, :, None, :]
        # [pair, head, d, n] -> [n, pair*2*D + head*D + d], + host-side bv
        out[b, :, g * HD : (g + 1) * HD] = (
            on.transpose(3, 0, 1, 2).reshape(N, HD) + bv[g * HD : (g + 1) * HD]
        )
    return out


_NC_CACHE = None


def _get_nc():
    global _NC_CACHE
    if _NC_CACHE is None:
        _NC_CACHE = build_nc()
    return _NC_CACHE


def kernel(**inputs):
    nc = _get_nc()
    in_maps = shard_inputs(inputs)
    res = run_bass_kernel_spmd(
        nc,
        in_maps,
        core_ids=list(range(NCORES)),
        trace=bool(int(os.environ.get("KERNEL_TRACE", "0"))),
    )
    return assemble(res.results, inputs, B=int(np.asarray(inputs["x"]).shape[0]))


# revision 48
# speedup vs baseline: 1.2137x; 1.0108x over previous
"""Multi-head attention forward kernel for Trainium2 (8 NeuronCores).

Problem: B=2, N=2048, C=1024, H=16 heads, head_dim=64.
    q = x @ Wq.T + bq  (same for k, v)
    out = softmax(q k^T / sqrt(C)) v       (per head), re-merged to [B, N, C]

Sharding: core = (batch b, head-group g): b = core // 4, g = core % 4.
Each core computes 4 heads of one batch element. No collectives needed --
outputs are disjoint; host gathers and finishes with a cheap epilogue
(normalize by the row-sums, add the V bias, transpose).

v2 design notes (measured atoms from microbench):
  - Any 512-col MM "slot" (single, row-packed pair, col-packed pair) paces
    at ~259 ns back-to-back; LDWEIGHTS hides completely. PE total ~125 us.
  - ACT exp from PSUM runs ~1.18 ns/elem regardless of op size ->
    ACT busy floor ~155 us/core. ACT is THE bottleneck; everything else
    is scheduled to keep the exp stream gapless.
  - st ring-3 (stp bufs=3, 6 PSUM banks) so QK can run 2 steps ahead of
    exp; o_ps 1 bank; proj+sums share 1 bank (ppsum bufs=1).
  - V bias is softmax-invariant additive on the output -> applied on host;
    V evacuates via plain tensor_copy (cheaper DVE).
  - out_o shipped as bf16 (halves out-DMA, 2x DVE copy mode).
  - Granular input DMA (w's first, then xt in nb-major 512-col slices) so
    the first exp fires ~8 us in; V blocks 0-2 emitted in the prologue,
    V[s+2] per qb0 step s, kt/qt projection blocks as PE filler inside the
    ACT-bound window (emission order = scheduler priority).
Outputs: out_o [2, 128, N] bf16 (pair, head-major O^T rows, queries),
         out_s [2, 2, N] f32   (pair, head, query sums).
"""

import os
import sys

import ml_dtypes
import numpy as np

for _p in ("/opt/trn_rl_repo",):
    if _p not in sys.path:
        sys.path.insert(0, _p)

import concourse.bass as bass  # noqa: E402
import concourse.tile as tile  # noqa: E402
from concourse import bacc, mybir  # noqa: E402
from concourse.bass_utils import run_bass_kernel_spmd  # noqa: E402

N = 2048  # sequence length
C = 1024  # model dim
D = 64  # head dim
NH = 4  # heads per core
HD = NH * D  # 256 output channels per core
NCORES = 8
KB = N // 128  # 16 key chunks of 128
QB = N // 512  # 4 query blocks of 512
KC = C // 128  # 8 contraction chunks for projections
SCALE = 1.0 / 32.0  # 1 / sqrt(C)

F32 = mybir.dt.float32
BF16 = mybir.dt.bfloat16
FP16 = mybir.dt.float16


def build_kernel(tc, xt, wqt, wkt, wvt, out_o, out_s):
    nc = tc.nc
    Exp = mybir.ActivationFunctionType.Exp

    with (
        tc.tile_pool(name="res", bufs=1) as res,
        tc.tile_pool(name="ppsum", bufs=1, space="PSUM") as ppsum,
        tc.tile_pool(name="stp", bufs=3, space="PSUM") as stp,
        tc.tile_pool(name="opp", bufs=1, space="PSUM") as opp,
        tc.tile_pool(name="ptp", bufs=10) as ptp,
        tc.tile_pool(name="otp", bufs=2) as otp,
        tc.tile_pool(name="ssp", bufs=2) as ssp,
    ):
        # ---- resident SBUF tensors ----
        # W layout [128, 2 bias cols + (m, k, d) m-major weights]: the two
        # bias columns ride inside the same contiguous DMA (a standalone
        # [128,1] bias DMA is a 4-byte-packet storm that stalls the queue),
        # and the m-major order lets each head-pair half load separately.
        wq_flat = res.tile([128, 2 + 2048], BF16, tag="wq", name="wq")
        wk_flat = res.tile([128, 2 + 2048], BF16, tag="wk", name="wk")
        wv_flat = res.tile([128, 2048], BF16, tag="wv", name="wv")
        xt_all = res.tile([128, KC, N], BF16, tag="xt", name="xt")
        xt_sb = [xt_all[:, k, :] for k in range(KC)]
        wq_m = wq_flat[:, 2:].rearrange("p (m k d) -> p m k d", m=2, k=KC)
        wk_m = wk_flat[:, 2:].rearrange("p (m k d) -> p m k d", m=2, k=KC)
        wv_m = wv_flat.rearrange("p (m k d) -> p m k d", m=2, k=KC)
        bqf = res.tile([128, 2], F32, tag="bqf", name="bqf")
        bkf = res.tile([128, 2], F32, tag="bkf", name="bkf")
        bq_sb = [bqf[:, m : m + 1] for m in range(2)]
        bk_sb = [bkf[:, m : m + 1] for m in range(2)]
        qt_sb = [res.tile([128, N], BF16, tag=f"qt{m}", name=f"qt{m}") for m in range(2)]
        kt_sb = [res.tile([128, N], BF16, tag=f"kt{m}", name=f"kt{m}") for m in range(2)]
        v_sb = [res.tile([128, NH, D], FP16, tag=f"v{kb}", name=f"v{kb}") for kb in range(KB)]
        ones_sb = res.tile([128, 1], FP16, tag="ones", name="ones")
        warm_sb = res.tile([1, 2], F32, tag="warm", name="warm")
        # pair-0 strip-mode state: resident softmax-sum parity accumulators
        # (all 4 qbs live at once) and per-qb SBUF O accumulators that
        # collect 4-kb strip partials from the single o PSUM bank.
        ssum_sb = [
            [res.tile([128, 2, 512], FP16, tag=f"ss{q}{j}", name=f"ss{q}{j}")
             for j in range(2)]
            for q in range(QB)
        ]
        o_acc = [res.tile([128, 512], F32, tag=f"oa{q}", name=f"oa{q}")
                 for q in range(QB)]

        # ---- input DMAs, ordered by consumer deadline (HBM bandwidth is
        # shared by all 8 cores; the whole input set takes tens of us).
        # FEW, BIG descriptors: each DMA_DIRECT2D trigger costs ~600 ns on
        # the sync queue and >.30 queued descriptors stall on ring space,
        # delaying later transfers by ~10 us (and the resulting PE idle
        # re-throttles HAM to K=4/8).  Partition lines stay >=1KB. ----
        # Two hardware DGE queues (SP + Activation) run in parallel: the
        # critical xt nb0/nb1 stream goes on the scalar queue (idle until
        # the first exp anyway) while the W stream runs on sync, halving
        # the serial prefix before the first exp.
        xtr = xt.rearrange("(k p) n -> p k n", p=128)
        nc.scalar.dma_start(out=xt_all[:, 0:4, 0:512], in_=xtr[:, 0:4, 0:512])
        nc.scalar.dma_start(out=xt_all[:, 4:8, 0:512], in_=xtr[:, 4:8, 0:512])
        nc.scalar.dma_start(out=xt_all[:, :, 512:1024], in_=xtr[:, :, 512:1024])
        for half in range(2):
            lo, hi = 2 + half * 512, 2 + (half + 1) * 512
            nc.sync.dma_start(out=wq_flat[:, (0 if half == 0 else lo) : hi],
                              in_=wqt[:, (0 if half == 0 else lo) : hi])
            nc.sync.dma_start(out=wk_flat[:, (0 if half == 0 else lo) : hi],
                              in_=wkt[:, (0 if half == 0 else lo) : hi])
        nc.sync.dma_start(out=wv_flat[:, 0:1024], in_=wvt[:, 0:1024])
        for nb in range(2, QB):
            nsl = slice(nb * 512, (nb + 1) * 512)
            nc.sync.dma_start(out=xt_all[:, :, nsl], in_=xtr[:, :, nsl])
        # pair-1 halves: needed only from pair0-qb2 onwards
        nc.sync.dma_start(out=wq_flat[:, 2 + 1024 :], in_=wqt[:, 2 + 1024 :])
        nc.sync.dma_start(out=wk_flat[:, 2 + 1024 :], in_=wkt[:, 2 + 1024 :])
        nc.sync.dma_start(out=wv_flat[:, 1024:], in_=wvt[:, 1024:])
        nc.vector.memset(ones_sb[:], 1.0)
        # widen the in-DMA bf16 bias columns to f32 for tensor_scalar
        nc.vector.tensor_copy(out=bqf[:], in_=wq_flat[:, 0:2])
        nc.vector.tensor_copy(out=bkf[:], in_=wk_flat[:, 0:2])
        # warm up the ACT exp table while DMAs land
        nc.vector.memset(warm_sb[:], 0.0)
        nc.scalar.activation(out=warm_sb[:, 0:1], in_=warm_sb[:, 1:2], func=Exp)

        # Projection blocks are emitted in HALVES (4 contraction chunks
        # each) on consecutive steps so a block never overflows a single
        # ACT-bound step window and stalls the exp chain through the PE
        # FIFO.  The live PSUM tile is kept in `pending` between halves.
        pending = {}

        def proj_qk_half(which, m, nb, part):
            key = (which, m, nb)
            nsl = slice(nb * 512, (nb + 1) * 512)
            if part == 0:
                pending[key] = ppsum.tile([128, 512], F32, tag="qkps", name="qkps")
            ps = pending[key]
            w_m = wq_m if which == "q" else wk_m
            for k in range(part * 4, part * 4 + 4):
                nc.tensor.matmul(
                    out=ps[:],
                    lhsT=w_m[:, m, k, :],
                    rhs=xt_sb[k][:, nsl],
                    start=(k == 0),
                    stop=(k == KC - 1),
                )
            if part == 1:
                b_sb = (bq_sb if which == "q" else bk_sb)[m]
                t_sb = (qt_sb if which == "q" else kt_sb)[m]
                nc.vector.tensor_scalar_add(out=t_sb[:, nsl], in0=ps[:], scalar1=b_sb[:])
                del pending[key]

        def proj_qk_block(which, m, nb):
            proj_qk_half(which, m, nb, 0)
            proj_qk_half(which, m, nb, 1)

        def proj_v_block(kb, half):
            # one head-pair's V columns: pair-0's V is needed from the very
            # first PV steps, pair-1's only once pair 1 starts -> split so
            # qb0 carries half the V-projection load.
            vps = ppsum.tile([128, 128], F32, tag="qkps", name="vps")
            for k in range(KC):
                nc.tensor.matmul(
                    out=vps[:],
                    lhsT=xt_sb[k][:, kb * 128 : (kb + 1) * 128],
                    rhs=wv_m[:, half, k, :],
                    start=(k == 0),
                    stop=(k == KC - 1),
                )
            # V bias is applied on the host (softmax-invariant): plain copy.
            nc.vector.tensor_copy(
                out=v_sb[kb][:, 2 * half : 2 * half + 2, :],
                in_=vps[:].rearrange("p (h d) -> p h d", h=2),
            )

        def attn_strip(p, filler_map):
            """Pair-0 attention in 4-kb-strip x qb cells, ordered so early
            cells only consume xt nb0/nb1 -- the later x slices stream in
            behind the compute instead of stalling the exp chain."""
            cells = [(0, 0), (1, 0), (0, 1), (1, 1), (2, 0), (2, 1), (0, 2),
                     (1, 2), (2, 2), (3, 0), (0, 3), (3, 1), (1, 3), (3, 2),
                     (2, 3), (3, 3)]
            prev = None       # pending PV(+evac) closure from the last step
            fin_pending = None
            for (j, i) in cells:
                qb = i
                qsl = slice(qb * 512, (qb + 1) * 512)
                o_ps = opp.tile([128, 512], F32, tag="o", name="o")
                for t in range(4):
                    kb = 4 * j + t
                    ksl = slice(kb * 128, (kb + 1) * 128)
                    st = stp.tile([128, 2, 512], F32, tag="st", name="st")
                    for h in range(2):
                        hsl = slice(h * D, (h + 1) * D)
                        nc.tensor.matmul(
                            out=st[:, h, :],
                            lhsT=kt_sb[p][hsl, ksl],
                            rhs=qt_sb[p][hsl, qsl],
                            start=True,
                            stop=True,
                        )
                    pt = ptp.tile([128, 2, 512], FP16, tag="pt", name="pt")
                    nc.scalar.activation(out=pt[:], in_=st[:], func=Exp, scale=SCALE)
                    if t == 1 and fin_pending is not None:
                        fin_pending()
                        fin_pending = None
                    if prev is not None:
                        prev()
                        prev = None

                    def step_pv(kb=kb, qb=qb, pt=pt, o_ps=o_ps, t=t, j=j):
                        for h in range(2):
                            nc.tensor.matmul(
                                out=o_ps[h * D : (h + 1) * D, :],
                                lhsT=v_sb[kb][:, 2 * p + h, :],
                                rhs=pt[:, h, :],
                                start=(t == 0),
                                stop=(t == 3),
                                tile_position=(0, h * D),
                                skip_group_check=True,
                            )
                        # evac first: the next cell's o alloc waits on it,
                        # so it must not queue behind the ssum add on DVE
                        if t == 3:
                            if j == 0:
                                nc.vector.tensor_copy(out=o_acc[qb][:], in_=o_ps[:])
                            else:
                                nc.vector.tensor_add(
                                    out=o_acc[qb][:], in0=o_acc[qb][:], in1=o_ps[:]
                                )
                        sj = ssum_sb[qb][kb % 2]
                        if kb < 2:
                            nc.vector.tensor_copy(out=sj[:], in_=pt[:])
                        else:
                            nc.vector.tensor_add(out=sj[:], in0=sj[:], in1=pt[:])

                    prev = step_pv
                    for fn in filler_map.get((qb, kb), ()):
                        fn()

                if j == 3:
                    def strip_fin(qb=qb, qsl=qsl):
                        s_ps = ppsum.tile([33, 512], F32, tag="qkps", name="sps")
                        for h in range(2):
                            for j2 in range(2):
                                nc.tensor.matmul(
                                    out=s_ps[32 * h : 32 * h + 1, :],
                                    lhsT=ones_sb[:],
                                    rhs=ssum_sb[qb][j2][:, h, :],
                                    start=(j2 == 0),
                                    stop=(j2 == 1),
                                    tile_position=(0, 32 * h),
                                    skip_group_check=True,
                                )
                        ss = otp.tile([33, 512], F32, tag="ss", name="ss")
                        for h in range(2):
                            nc.vector.tensor_copy(
                                out=ss[32 * h : 32 * h + 1, :],
                                in_=s_ps[32 * h : 32 * h + 1, :],
                            )
                        ss_view = bass.AP(
                            tensor=ss.tensor, offset=ss.offset,
                            ap=[[32 * ss.ap[0][0], 2]] + list(ss.ap[1:]),
                        )
                        nc.sync.dma_start(out=out_s[p, :, qsl], in_=ss_view)
                        ot = otp.tile([128, 512], BF16, tag="ot", name="ot")
                        nc.vector.tensor_copy(out=ot[:], in_=o_acc[qb][:])
                        nc.sync.dma_start(out=out_o[p, :, qsl], in_=ot[:])

                    fin_pending = strip_fin
            prev()
            return fin_pending

        def attn(p, filler_hook=None, carry_fin=None):
            fin_pending = carry_fin
            for qb in range(QB):
                qsl = slice(qb * 512, (qb + 1) * 512)
                # both heads' O^T col-packed: head h at partitions h*64..
                o_ps = opp.tile([128, 512], F32, tag="o", name="o")
                # running sums of P^T chunks (softmax denominators): two
                # fp16 parity accumulators keep the DVE in its fast 2-byte
                # mode and halve the accumulation depth.
                ssum = [
                    ssp.tile([128, 2, 512], FP16, tag=f"ssum{j}", name=f"ssum{j}")
                    for j in range(2)
                ]

                def emit_pv(args):
                    kb, pt = args
                    for h in range(2):
                        nc.tensor.matmul(
                            out=o_ps[h * D : (h + 1) * D, :],
                            lhsT=v_sb[kb][:, 2 * p + h, :],
                            rhs=pt[:, h, :],
                            start=(kb == 0),
                            stop=(kb == KB - 1),
                            tile_position=(0, h * D),
                            skip_group_check=True,
                        )
                    sj = ssum[kb % 2]
                    if kb < 2:
                        nc.vector.tensor_copy(out=sj[:], in_=pt[:])
                    else:
                        nc.vector.tensor_add(out=sj[:], in0=sj[:], in1=pt[:])

                # Per step: QK -> exp -> PV(prev) -> fillers.  The QK/exp
                # chain leads; PV lags one step (pt pool decouples); filler
                # projection blocks absorb the PE slack under the ACT-bound
                # exp stream.
                prev = None
                for kb in range(KB):
                    ksl = slice(kb * 128, (kb + 1) * 128)
                    # st layout [128 keys, head, 512 q] fp32: head h
                    # occupies its own PSUM bank; ring-3 lets QK run ~2
                    # steps ahead of the exp stream.
                    st = stp.tile([128, 2, 512], F32, tag="st", name="st")
                    for h in range(2):
                        hsl = slice(h * D, (h + 1) * D)
                        nc.tensor.matmul(
                            out=st[:, h, :],
                            lhsT=kt_sb[p][hsl, ksl],
                            rhs=qt_sb[p][hsl, qsl],
                            start=True,
                            stop=True,
                        )
                    pt = ptp.tile([128, 2, 512], FP16, tag="pt", name="pt")
                    nc.scalar.activation(out=pt[:], in_=st[:], func=Exp, scale=SCALE)
                    if kb == 1 and fin_pending is not None:
                        fin_pending()
                        fin_pending = None
                    if prev is not None:
                        emit_pv(prev)
                    prev = (kb, pt)
                    if filler_hook is not None:
                        filler_hook(qb, kb)
                emit_pv(prev)

                # Finalize (partition-reduce the running sums with
                # ones-vector matmuls -- both parity accumulators accumulate
                # into the same PSUM row, head h at PSUM partition 32*h --
                # then evacuate sums + O and DMA out).  Deferred into the
                # next qb's step 1 so it never sits ahead of the next qb's
                # QK chain in the engine FIFOs.
                def finalize(qb=qb, qsl=qsl, o_ps=o_ps, ssum=ssum):
                    s_ps = ppsum.tile([33, 512], F32, tag="qkps", name="sps")
                    for h in range(2):
                        for j in range(2):
                            nc.tensor.matmul(
                                out=s_ps[32 * h : 32 * h + 1, :],
                                lhsT=ones_sb[:],
                                rhs=ssum[j][:, h, :],
                                start=(j == 0),
                                stop=(j == 1),
                                tile_position=(0, 32 * h),
                                skip_group_check=True,
                            )
                    ss = otp.tile([33, 512], F32, tag="ss", name="ss")
                    for h in range(2):
                        nc.vector.tensor_copy(
                            out=ss[32 * h : 32 * h + 1, :],
                            in_=s_ps[32 * h : 32 * h + 1, :],
                        )
                    ss_view = bass.AP(
                        tensor=ss.tensor, offset=ss.offset,
                        ap=[[32 * ss.ap[0][0], 2]] + list(ss.ap[1:]),
                    )
                    nc.sync.dma_start(out=out_s[p, :, qsl], in_=ss_view)
                    ot = otp.tile([128, 512], BF16, tag="ot", name="ot")
                    nc.vector.tensor_copy(out=ot[:], in_=o_ps[:])
                    nc.sync.dma_start(out=out_o[p, :, qsl], in_=ot[:])

                fin_pending = finalize
            return fin_pending

        def proj_qk_first():
            qps = ppsum.tile([128, 512], F32, tag="qkps", name="qkps")
            kps = ppsum.tile([128, 512], F32, tag="qkps", name="qkps")
            for k in range(KC):
                for w_m, ps in ((wq_m, qps), (wk_m, kps)):
                    nc.tensor.matmul(
                        out=ps[:],
                        lhsT=w_m[:, 0, k, :],
                        rhs=xt_sb[k][:, 0:512],
                        start=(k == 0),
                        stop=(k == KC - 1),
                    )
            nc.vector.tensor_scalar_add(out=qt_sb[0][:, 0:512], in0=qps[:], scalar1=bq_sb[0][:])
            nc.vector.tensor_scalar_add(out=kt_sb[0][:, 0:512], in0=kps[:], scalar1=bk_sb[0][:])

        # Filler schedule.  Pair-0 (strip mode) keys fillers by the actual
        # (qb, kb) of each step; blocks are placed after their input DMA
        # lands (nb1 ~30us, nb2 ~38, nb3 ~46, wv-m1 ~52) and at least one
        # cell before their consumer.
        def sched(table, qb, kb):
            for (q, s), fn in table:
                if q == qb and s == kb:
                    fn()

        def V0(kb):
            return lambda: proj_v_block(kb, 0)

        def V1(kb):
            return lambda: proj_v_block(kb, 1)

        def PQ(which, m, nb, part):
            return lambda: proj_qk_half(which, m, nb, part)

        p0_map = {
            (0, 0): [V0(1)], (0, 1): [V0(2)],
            # kt-nb1 halves both on the cell's LAST slot: they stall on the
            # nb1 DMA, and anywhere earlier they block the cell's own QKs
            # through the PE FIFO.
            (0, 3): [PQ("k", 0, 1, 0), PQ("k", 0, 1, 1), V0(3)],
            (0, 4): [PQ("q", 0, 1, 0), V0(4)], (0, 5): [PQ("q", 0, 1, 1), V0(5)],
            (0, 6): [V0(6)], (0, 7): [V0(7)],
            (1, 2): [PQ("k", 0, 2, 0)], (1, 3): [PQ("k", 0, 2, 1)],
            (0, 8): [V0(8)], (0, 9): [V0(9)], (0, 10): [V0(10)], (0, 11): [V0(11)],
            (1, 6): [PQ("q", 0, 2, 0)], (1, 7): [PQ("q", 0, 2, 1)],
            (2, 0): [PQ("q", 0, 3, 0)], (2, 1): [PQ("q", 0, 3, 1)],
            (2, 4): [V1(0)], (2, 5): [PQ("k", 0, 3, 0)],
            (2, 6): [V1(1)], (2, 7): [PQ("k", 0, 3, 1)],
            (2, 8): [V1(2)], (2, 10): [V1(3)],
            (0, 12): [V0(12)], (0, 13): [V0(13)], (0, 14): [V0(14)], (0, 15): [V0(15)],
            (3, 0): [V1(4)], (3, 1): [V1(5)],
            (3, 2): [PQ("k", 1, 0, 0)], (3, 3): [PQ("k", 1, 0, 1)],
            (3, 10): [V1(8)], (3, 11): [V1(9)],
            (3, 14): [V1(6)], (3, 15): [V1(7)],
            (3, 4): [V1(10)], (3, 7): [V1(11)],
            (3, 5): [PQ("q", 1, 0, 0)], (3, 6): [PQ("q", 1, 0, 1)],
            (2, 12): [PQ("k", 1, 1, 0)], (2, 13): [PQ("k", 1, 1, 1)],
            (2, 14): [V1(12)], (2, 15): [V1(13)],
            (3, 8): [V1(14)], (3, 9): [V1(15)],
        }

        p1_table = [
            ((0, 2), lambda: proj_qk_half("k", 1, 2, 0)),
            ((0, 3), lambda: proj_qk_half("k", 1, 2, 1)),
            ((0, 6), lambda: proj_qk_half("k", 1, 3, 0)),
            ((0, 7), lambda: proj_qk_half("k", 1, 3, 1)),
            ((0, 10), lambda: proj_qk_half("q", 1, 1, 0)),
            ((0, 11), lambda: proj_qk_half("q", 1, 1, 1)),
            ((1, 1), lambda: proj_qk_half("q", 1, 2, 0)),
            ((1, 2), lambda: proj_qk_half("q", 1, 2, 1)),
            ((1, 7), lambda: proj_qk_half("q", 1, 3, 0)),
            ((1, 8), lambda: proj_qk_half("q", 1, 3, 1)),
        ]

        proj_qk_first()
        proj_v_block(0, 0)
        fin = attn_strip(0, p0_map)
        fin = attn(1, filler_hook=lambda qb, kb: sched(p1_table, qb, kb),
                   carry_fin=fin)
        fin()


def build_nc():
    nc = bacc.Bacc(
        "TRN2",
        target_bir_lowering=False,
        debug=False,
        num_devices=NCORES,
        enable_partition_id=False,
    )
    xt = nc.dram_tensor("xt", [C, N], BF16, kind="ExternalInput").ap()
    wqt = nc.dram_tensor("wqt", [128, 2 + 2048], BF16, kind="ExternalInput").ap()
    wkt = nc.dram_tensor("wkt", [128, 2 + 2048], BF16, kind="ExternalInput").ap()
    wvt = nc.dram_tensor("wvt", [128, 2048], BF16, kind="ExternalInput").ap()
    out_o = nc.dram_tensor("out_o", [2, 128, N], BF16, kind="ExternalOutput").ap()
    out_s = nc.dram_tensor("out_s", [2, 2, N], F32, kind="ExternalOutput").ap()

    with tile.TileContext(nc) as tc:
        build_kernel(tc, xt, wqt, wkt, wvt, out_o, out_s)
    nc.compile()
    return nc


def _w_prep(w, sl, bias=None):
    # [HD-slice, C] weight -> SBUF-ready m-major [128, (2 kd)]: element
    # (c=k*128+p, h=m*128+j) -> [p, m, k, j], flattened; with the two bias
    # columns (bias[m*128+p] on partition p) prepended when given.
    wt = np.asarray(w, np.float32)[sl, :].T  # [C, HD]
    wt = wt.reshape(KC, 128, 2, 128).transpose(1, 2, 0, 3).reshape(128, 2048)
    if bias is None:
        return np.ascontiguousarray(wt).astype(ml_dtypes.bfloat16)
    b = np.asarray(bias, np.float32)[sl].reshape(2, 128).T  # [128, 2]
    return np.ascontiguousarray(np.concatenate([b, wt], axis=1)).astype(
        ml_dtypes.bfloat16
    )


def shard_inputs(inputs):
    x = np.asarray(inputs["x"], np.float32)
    in_maps = []
    for core in range(NCORES):
        b, g = core // 4, core % 4
        sl = slice(g * HD, (g + 1) * HD)
        in_maps.append(
            {
                "xt": np.ascontiguousarray(x[b].T).astype(ml_dtypes.bfloat16),
                "wqt": _w_prep(inputs["Wq"], sl, inputs["bq"]),
                "wkt": _w_prep(inputs["Wk"], sl, inputs["bk"]),
                "wvt": _w_prep(inputs["Wv"], sl),
            }
        )
    return in_maps


def assemble(results, inputs, B=2):
    bv = np.asarray(inputs["bv"], np.float32)
    out = np.zeros((B, N, C), np.float32)
    for core in range(NCORES):
        b, g = core // 4, core % 4
        oo = np.asarray(results[core]["out_o"], np.float32)  # [2, 128, N]
        os_ = np.asarray(results[core]["out_s"], np.float32)  # [2, 2, N]
        o = oo.reshape(2, 2, D, N)  # [pair, head, d, n]
        on = o / os_[:, :, None, :]
        # [pair, head, d, n] -> [n, pair*2*D + head*D + d], + host-side bv
        out[b, :, g * HD : (g + 1) * HD] = (
            on.transpose(3, 0, 1, 2).reshape(N, HD) + bv[g * HD : (g + 1) * HD]
        )
    return out


_NC_CACHE = None


def _get_nc():
    global _NC_CACHE
    if _NC_CACHE is None:
        _NC_CACHE = build_nc()
    return _NC_CACHE


def kernel(**inputs):
    nc = _get_nc()
    in_maps = shard_inputs(inputs)
    res = run_bass_kernel_spmd(
        nc,
        in_maps,
        core_ids=list(range(NCORES)),
        trace=bool(int(os.environ.get("KERNEL_TRACE", "0"))),
    )
    return assemble(res.results, inputs, B=int(np.asarray(inputs["x"]).shape[0]))
